# revision 22
# baseline (speedup 1.0000x reference)
"""Trainium2 Bass kernel for a transformer encoder layer (nn_Encoder).

x:[2,2048,1024] f32, 8 NeuronCores. Hybrid data/tensor parallel: core c
handles batch n=c//4 and head-group r=c%4 (4 of 16 heads). Each core
computes Q/K/V for its 4 heads over all 2048 tokens (no K/V recompute
redundancy), runs attention per 512-query quarter, then a partial output
projection; a per-quarter ReduceScatter over the 4-core group sums the
partials and hands each core 128 tokens per quarter (512 total) for the
LN1+FFN+LN2 tail. The 4 ReduceScatters run on the collective cores,
overlapped with attention of later quarters.

Matmul operands are bf16 (same PE rate as f32r, half the DMA/SBUF);
accumulation and the collective stay f32. LN stats in f32.
"""
import os
import sys

for _p in ("/opt/trn_rl_repo", "/root/.axon_site/_ro/trn_rl_repo"):
    if os.path.isdir(_p) and _p not in sys.path:
        sys.path.insert(0, _p)

import numpy as np
import ml_dtypes
import concourse.bass as bass
import concourse.mybir as mybir
import concourse.tile as tile
from concourse import bacc
from concourse.bass_utils import run_bass_kernel_spmd
from concourse.masks import make_identity

F32 = mybir.dt.float32
BF16 = mybir.dt.bfloat16
AF = mybir.ActivationFunctionType
ALU = mybir.AluOpType
BFNP = ml_dtypes.bfloat16

D = 1024
H = 16
HD = 64
FF = 4096
L = 2048
NB = 2
P = 128
DC = D // P       # 8 chunks of the model dim
KT = L // P       # 16 key tiles
FC = FF // P      # 32 ff chunks
NP = 2            # head pairs per core (4 heads)
QD = 4            # query quarters
QL = L // QD      # 512 queries per quarter
EPS = 1e-5
GROUPS = [[0, 1, 2, 3], [4, 5, 6, 7]]

_CACHED_NC = {}


def _layernorm(nc, pool, dst, src, g_t, be_t, eps_t, zero_t, affine):
    """dst = (src - mean)/sqrt(var + eps) [* g + be], row-wise over 1024.

    var = E[x^2] - mu^2 (safe here: |mu| << rms). One Newton step refines
    the reciprocal sqrt.
    """
    mu = pool.tile([P, 1], F32, tag="ln_mu")
    nc.vector.tensor_reduce(mu[:], src, mybir.AxisListType.X, ALU.add)
    nc.vector.tensor_scalar_mul(mu[:], mu[:], 1.0 / D)
    c = pool.tile([P, D], F32, tag="ln_c")
    ss = pool.tile([P, 1], F32, tag="ln_ss")
    nc.scalar.activation(c[:], src, AF.Square, accum_out=ss[:])
    vv = pool.tile([P, 1], F32, tag="ln_v")
    nc.vector.tensor_scalar(vv[:], ss[:], 1.0 / D, EPS, ALU.mult, ALU.add)
    m2 = pool.tile([P, 1], F32, tag="ln_m2")
    nc.vector.tensor_tensor(m2[:], mu[:], mu[:], ALU.mult)
    nc.vector.tensor_tensor(vv[:], vv[:], m2[:], ALU.subtract)
    s = pool.tile([P, 1], F32, tag="ln_s")
    nc.scalar.activation(s[:], vv[:], AF.Sqrt, bias=zero_t[:])
    r = pool.tile([P, 1], F32, tag="ln_r")
    nc.vector.reciprocal(r[:], s[:])
    t = pool.tile([P, 1], F32, tag="ln_t")
    nc.vector.tensor_tensor(t[:], r[:], r[:], ALU.mult)
    nc.vector.tensor_tensor(t[:], t[:], vv[:], ALU.mult)
    nc.vector.tensor_scalar(t[:], t[:], -0.5, 1.5, ALU.mult, ALU.add)
    nc.vector.tensor_tensor(r[:], r[:], t[:], ALU.mult)
    nc.vector.tensor_scalar(c[:], src, mu[:], None, ALU.subtract)
    nc.vector.tensor_scalar(dst, c[:], r[:], None, ALU.mult)
    if affine:
        nc.vector.tensor_tensor(dst, dst, g_t[:], ALU.mult)
        nc.vector.tensor_tensor(dst, dst, be_t[:], ALU.add)


def _build_nc(affine=True):
    nc = bacc.Bacc("TRN2", target_bir_lowering=False, num_devices=8)

    def dparam(name, shape, dt=BF16):
        return nc.dram_tensor(name, shape, dt, kind="ExternalInput")

    xT = dparam("xT", [P, DC, L])           # x[n].T as [p, dc, t] (d=dc*128+p)
    xq = dparam("xq", [P, QD, D], F32)      # owned token tiles, + b_o folded
    wq = dparam("wq", [P, NP, DC, P])       # [dpart, pair, dchunk, qcols]
    wk = dparam("wk", [P, NP, DC, P])
    wv = dparam("wv", [P, DC, 2 * P])       # [dpart, dchunk, vcols(4 heads)]
    wo = dparam("wo", [P, NP, D])           # [hd-part, pair, ocols]
    w1 = dparam("w1", [FC // 4, P, 4, DC, P])  # per-fc4 chunk, SBUF layout
    w2 = dparam("w2", [FC // 4, P, 4, D])
    bq = dparam("bq", [P, NP], F32)
    bk = dparam("bk", [P, NP], F32)
    bv = dparam("bv", [P, 2 * P], F32)      # per-column bias, broadcast rows
    b1 = dparam("b1", [P, FC], F32)
    b2b = dparam("b2b", [P, D], F32)
    g1b = dparam("g1b", [P, D], F32)
    be1b = dparam("be1b", [P, D], F32)
    g2b = dparam("g2b", [P, D], F32)
    be2b = dparam("be2b", [P, D], F32)
    vones = dparam("vones", [P, KT], F32)

    parts = [nc.dram_tensor(f"part{q}", [QL, D], F32) for q in range(QD)]
    reds = [nc.dram_tensor(f"red{q}", [P, D], F32) for q in range(QD)]
    y = nc.dram_tensor("y", [QD, P, D], F32, kind="ExternalOutput")

    with tile.TileContext(nc) as tc:
        with tc.tile_pool(name="pers", bufs=1) as pers, \
             tc.tile_pool(name="wp", bufs=1) as wp, \
             tc.tile_pool(name="stg", bufs=2) as stg, \
             tc.tile_pool(name="atp", bufs=3) as atp, \
             tc.tile_pool(name="atd", bufs=2) as atd, \
             tc.tile_pool(name="hpp", bufs=2) as hpp, \
             tc.tile_pool(name="lnp", bufs=2) as lnp, \
             tc.tile_pool(name="fp", bufs=2) as fp, \
             tc.tile_pool(name="fp2", bufs=2) as fp2, \
             tc.tile_pool(name="t2p", bufs=2) as t2p, \
             tc.tile_pool(name="ps", bufs=1, space="PSUM") as ps:

            # ---- persistent SBUF ----
            xT_t = pers.tile([P, DC, L], BF16, tag="blobA")      # 32KB
            qT_t = pers.tile([P, NP, L], BF16, tag="qT")
            kT_t = pers.tile([P, NP, L], BF16, tag="kT")
            v_aug = pers.tile([P, KT, 4, HD + 1], BF16, tag="vaug")
            outSB = pers.tile([P, NP, QL], BF16, tag="outSB")
            h_t = pers.tile([P, QD, D], F32, tag="h")            # post-LN1
            hT_t = pers.tile([P, DC, QD * P], BF16, tag="hT")
            xq_t = pers.tile([P, QD, D], F32, tag="xq")
            eps_t = pers.tile([P, 1], F32, tag="eps")
            zero_t = pers.tile([P, 1], F32, tag="zero")
            ident = pers.tile([P, P], F32, tag="ident")
            nc.gpsimd.memset(eps_t[:], EPS)
            nc.gpsimd.memset(zero_t[:], 0.0)
            make_identity(nc, ident[:])

            # psum tiles: fixed 8-bank plan, manually assigned per phase
            def pt(tag, name, cols=1024):
                return ps.tile([P, cols], F32, tag=tag, name=name)

            # ---- weight / bias prefetch (scalar queue) ----
            wq_t = wp.tile([P, NP, DC, P], BF16, tag="wq")
            wk_t = wp.tile([P, NP, DC, P], BF16, tag="wk")
            wv_t = wp.tile([P, DC, 2 * P], BF16, tag="wv")
            wo_t = wp.tile([P, NP, D], BF16, tag="wo")
            bq_t = wp.tile([P, NP], F32, tag="bq")
            bk_t = wp.tile([P, NP], F32, tag="bk")
            bv_t = wp.tile([P, 2 * P], F32, tag="bv")
            b1_t = wp.tile([P, FC], F32, tag="b1")
            b2b_t = wp.tile([P, D], F32, tag="b2b")
            ones_t = wp.tile([P, KT], F32, tag="ones")
            nc.scalar.dma_start(wq_t[:], wq[:])
            nc.scalar.dma_start(wk_t[:], wk[:])
            nc.scalar.dma_start(wv_t[:], wv[:])
            nc.scalar.dma_start(wo_t[:], wo[:])
            nc.scalar.dma_start(bq_t[:], bq[:])
            nc.scalar.dma_start(bk_t[:], bk[:])
            nc.scalar.dma_start(bv_t[:], bv[:])
            nc.scalar.dma_start(b1_t[:], b1[:])
            nc.scalar.dma_start(b2b_t[:], b2b[:])
            nc.scalar.dma_start(ones_t[:], vones[:])
            if affine:
                g1b_t = wp.tile([P, D], F32, tag="g1b")
                be1b_t = wp.tile([P, D], F32, tag="be1b")
                g2b_t = wp.tile([P, D], F32, tag="g2b")
                be2b_t = wp.tile([P, D], F32, tag="be2b")
                nc.scalar.dma_start(g1b_t[:], g1b[:])
                nc.scalar.dma_start(be1b_t[:], be1b[:])
                nc.scalar.dma_start(g2b_t[:], g2b[:])
                nc.scalar.dma_start(be2b_t[:], be2b[:])
            else:
                g1b_t = be1b_t = g2b_t = be2b_t = None

            # x stream (sync queue), per-dc so Q proj starts on chunk 0
            for dc in range(DC):
                nc.sync.dma_start(xT_t[:, dc, :], xT[:, dc, :])
            nc.sync.dma_start(xq_t[:], xq[:])

            # 8 half-bank accumulation slots for the projection phases.
            # One tile per tag per phase; slots address halves explicitly
            # (repeated same-tag tile creation would WAW-serialize).
            def phase_slots(phase):
                tiles = {t: pt(t, f"{phase}_{t}",
                               cols=512 if t in ("pE", "pF") else 1024)
                         for t in ("pAB", "pCD", "pGH", "pE", "pF")}
                layout = [("pAB", 0), ("pAB", 512), ("pCD", 0), ("pCD", 512),
                          ("pGH", 0), ("pGH", 512), ("pE", 0), ("pF", 0)]
                return [(tiles[t], off) for t, off in layout]

            # ================= Q projection (dc-outer sweeps) ==========
            # dc-outer so the first matmuls ride the xT chunk stream.
            qsl = phase_slots("q")
            for pair in range(NP):
                for dc in range(DC):
                    for qt in range(4):
                        tile_, off = qsl[pair * 4 + qt]
                        nc.tensor.matmul(tile_[:, off:off + 512],
                                         wq_t[:, pair, dc, :],
                                         xT_t[:, dc, qt * 512:(qt + 1) * 512],
                                         start=(dc == 0), stop=(dc == DC - 1))
                for qt in range(4):
                    tile_, off = qsl[pair * 4 + qt]
                    nc.vector.tensor_scalar(
                        qT_t[:, pair, qt * 512:(qt + 1) * 512],
                        tile_[:, off:off + 512],
                        bq_t[:, pair:pair + 1], None, ALU.add)

            # ================= K projection (dc-inner) =================
            ksl = phase_slots("k")
            for pair in range(NP):
                for qt in range(4):
                    i = pair * 4 + qt
                    tile_, off = ksl[i]
                    for dc in range(DC):
                        nc.tensor.matmul(tile_[:, off:off + 512],
                                         wk_t[:, pair, dc, :],
                                         xT_t[:, dc, qt * 512:(qt + 1) * 512],
                                         start=(dc == 0), stop=(dc == DC - 1))
                    nc.vector.tensor_scalar(
                        kT_t[:, pair, qt * 512:(qt + 1) * 512],
                        tile_[:, off:off + 512],
                        bk_t[:, pair:pair + 1], None, ALU.add)

            # ================= V projection ============================
            nc.vector.tensor_copy(
                v_aug[:, :, :, HD],
                ones_t[:, :, None].to_broadcast([P, KT, 4]))
            vsl = phase_slots("v")
            for tt in range(KT):
                tile_, off = vsl[tt % 8]
                for dc in range(DC):
                    nc.tensor.matmul(tile_[:, off:off + 256],
                                     xT_t[:, dc, tt * P:(tt + 1) * P],
                                     wv_t[:, dc, :],
                                     start=(dc == 0), stop=(dc == DC - 1))
                nc.vector.tensor_tensor(
                    v_aug[:, tt, :, 0:HD],
                    tile_[:, off:off + 256].rearrange("p (h c) -> p h c", c=HD),
                    bv_t[:].rearrange("p (h c) -> p h c", c=HD),
                    ALU.add)

            # ====== attention + out-proj partial + RS, per quarter =====
            def post_quarter(qd):
                """RS result -> +x residual -> LN1 (transpose issued apart)."""
                hpre = hpp.tile([P, D], F32, tag="hpre", name=f"hpre_{qd}")
                nc.sync.dma_start(hpre[:], reds[qd][:])
                nc.vector.tensor_tensor(hpre[:], hpre[:], xq_t[:, qd, :],
                                        ALU.add)
                _layernorm(nc, lnp, h_t[:, qd, :], hpre[:], g1b_t, be1b_t,
                           eps_t, zero_t, affine)

            def pe_transpose(qd):
                """h_t[:, qd, :] -> hT_t[:, :, qd*P:...] via 8 PE transposes
                through a pGH psum tile, drained by the Pool engine."""
                trp = pt("pGH", f"tr_{qd}")
                for dc in range(DC):
                    nc.tensor.transpose(trp[:, dc * P:(dc + 1) * P],
                                        h_t[:, qd, dc * P:(dc + 1) * P],
                                        ident[:])
                nc.vector.tensor_copy(
                    hT_t[:, :, qd * P:(qd + 1) * P],
                    trp[:].rearrange("p (c t) -> p c t", t=P))

            for qd in range(QD):
                qcols = slice(qd * QL, (qd + 1) * QL)
                # ---- attention for 2 pairs ----
                stprot = ["pAB", "pCD"]
                sti = 0
                for pair in range(NP):
                    pvs = [pt("pE", f"pv_{qd}_{pair}_0", cols=512),
                           pt("pF", f"pv_{qd}_{pair}_1", cols=512)]
                    for ktp in range(KT // 2):
                        # S for both heads first, then both PVs; the stp psum
                        # rotates over 3 double-bank slots so the PE can run
                        # several S groups ahead of the ACT exp stream.
                        ptts = []
                        for h2 in range(2):
                            rows = slice(h2 * HD, h2 * HD + HD)
                            stp = pt(stprot[sti % 2],
                                     f"st_{qd}_{pair}_{ktp}_{h2}")
                            sti += 1
                            for j in range(2):
                                kt = 2 * ktp + j
                                nc.tensor.matmul(
                                    stp[:, j * 512:(j + 1) * 512],
                                    kT_t[rows, pair, kt * P:(kt + 1) * P],
                                    qT_t[rows, pair, qcols],
                                    start=True, stop=True)
                            ptt = atp.tile([P, 2, 512], BF16, tag="pt",
                                           name=f"pt_{qd}_{pair}_{ktp}_{h2}")
                            nc.scalar.activation(
                                ptt[:],
                                stp[:].rearrange("p (c n) -> p c n", n=512),
                                AF.Exp, scale=0.125)
                            ptts.append(ptt)
                        for h2 in range(2):
                            for j in range(2):
                                kt = 2 * ktp + j
                                nc.tensor.matmul(
                                    pvs[h2][:HD + 1, :],
                                    v_aug[:, kt, 2 * pair + h2, :],
                                    ptts[h2][:, j, :],
                                    start=(kt == 0), stop=(kt == KT - 1))
                    for h2 in range(2):
                        rows = slice(h2 * HD, h2 * HD + HD)
                        den = atd.tile([1, QL], F32, tag="den",
                                       name=f"den_{qd}_{pair}_{h2}")
                        nc.vector.reciprocal(den[:], pvs[h2][HD:HD + 1, :])
                        denb = atd.tile([HD, QL], F32, tag="denb",
                                        name=f"denb_{qd}_{pair}_{h2}")
                        nc.gpsimd.partition_broadcast(denb[:], den[:])
                        nc.vector.tensor_tensor(outSB[rows, pair, :],
                                                pvs[h2][:HD, :], denb[:],
                                                ALU.mult)

                # ---- out-proj partial: [512 q, 1024], psum->SBUF->DRAM ----
                poslots = ["pGH", "pAB", "pCD", "pGH"]
                for tt4 in range(4):
                    po = pt(poslots[tt4], f"po_{qd}_{tt4}")
                    for pair in range(NP):
                        for oc in range(2):
                            nc.tensor.matmul(
                                po[:, oc * 512:(oc + 1) * 512],
                                outSB[:, pair, tt4 * P:(tt4 + 1) * P],
                                wo_t[:, pair, oc * 512:(oc + 1) * 512],
                                start=(pair == 0), stop=(pair == NP - 1))
                    pst = stg.tile([P, D], F32, tag="pstg",
                                   name=f"pstg_{qd}_{tt4}")
                    nc.vector.tensor_copy(pst[:], po[:])
                    nc.sync.dma_start(
                        parts[qd].rearrange("(a p) d -> p a d", p=P)[:, tt4, :],
                        pst[:])
                nc.gpsimd.collective_compute(
                    "ReduceScatter", ALU.add, replica_groups=GROUPS,
                    ins=[parts[qd][:].opt()], outs=[reds[qd][:].opt()])
            # posts AFTER the full attention loop, with scheduler hints so
            # the greedy per-engine scheduler does not slot their long
            # collective-gated waits into the attention instruction streams
            # (a wait at a queue head blocks everything behind it).
            for qd in range(QD):
                with tc.tile_wait_until(0.160 + 0.045 * qd):
                    post_quarter(qd)
            for qd in range(QD - 1):
                pe_transpose(qd)

            # ================= FFN =====================================
            # Four phases: fc1(c1) -> fc2(c1) -> fc1(c2) -> fc2(c2).
            # fc2(c1) only needs ff1 for chunk-1 tokens, so it covers the
            # RS_q3 -> LN1_q3 -> transpose latency that gates fc1(c2).
            # w1 and w2 each stream twice (bf16, ~270 GB/s bursts).
            ff1 = pers.tile([P, FC, QD * P], BF16, tag="blobA")

            def fc1_chunk(ch):
                tcols = slice(ch * 256, (ch + 1) * 256)
                f1AB = pt("pAB", f"f1AB_{ch}")
                f1CD = pt("pCD", f"f1CD_{ch}")
                f1slots = [(f1AB, 0), (f1AB, 512), (f1CD, 0), (f1CD, 512)]
                for fc4 in range(FC // 4):
                    w1_t = fp.tile([P, 4, DC, P], BF16, tag="w1s")
                    nc.sync.dma_start(w1_t[:], w1[fc4])
                    for f in range(4):
                        fc = 4 * fc4 + f
                        tile_, sl = f1slots[f]
                        for dc in range(DC):
                            nc.tensor.matmul(
                                tile_[:, sl:sl + 256],
                                w1_t[:, f, dc, :], hT_t[:, dc, tcols],
                                start=(dc == 0), stop=(dc == DC - 1))
                        nc.vector.tensor_scalar(
                            ff1[:, fc, tcols], tile_[:, sl:sl + 256],
                            b1_t[:, fc:fc + 1], 0.0, ALU.add, ALU.max)

            def fc2_chunk(ch):
                tts = (2 * ch, 2 * ch + 1)
                tGH = pt("pGH", f"f2GH_{ch}")
                tE = pt("pE", f"f2E_{ch}", cols=512)
                tF = pt("pF", f"f2F_{ch}", cols=512)
                dsts = {(tts[0], 0): tGH[:, 0:512], (tts[0], 1): tGH[:, 512:1024],
                        (tts[1], 0): tE[:, :], (tts[1], 1): tF[:, :]}
                for fc4 in range(FC // 4):
                    w2_t = fp2.tile([P, 4, D], BF16, tag="w2s")
                    nc.sync.dma_start(w2_t[:], w2[fc4])
                    for f in range(4):
                        fc = 4 * fc4 + f
                        st = (fc == 0)
                        sp = (fc == FC - 1)
                        for tt in tts:
                            for oc in range(2):
                                nc.tensor.matmul(
                                    dsts[(tt, oc)],
                                    ff1[:, fc, tt * P:(tt + 1) * P],
                                    w2_t[:, f, oc * 512:(oc + 1) * 512],
                                    start=st, stop=sp)
                for i, tt in enumerate(tts):
                    t2 = t2p.tile([P, D], F32, tag="t2", name=f"t2_{tt}")
                    if i == 0:
                        nc.vector.tensor_tensor(t2[:], tGH[:],
                                                h_t[:, tt, :], ALU.add)
                    else:
                        nc.vector.tensor_tensor(t2[:, 0:512], tE[:],
                                                h_t[:, tt, 0:512], ALU.add)
                        nc.vector.tensor_tensor(t2[:, 512:1024], tF[:],
                                                h_t[:, tt, 512:1024], ALU.add)
                    nc.vector.tensor_tensor(t2[:], t2[:], b2b_t[:], ALU.add)
                    _layernorm(nc, lnp, t2[:], t2[:], g2b_t, be2b_t, eps_t,
                               zero_t, affine)
                    nc.sync.dma_start(y[tt], t2[:])

            fc1_chunk(0)
            fc2_chunk(0)
            pe_transpose(QD - 1)
            fc1_chunk(1)
            fc2_chunk(1)

    nc.compile()
    return nc


def make_in_maps(x, w_qkv, b_qkv, w_o, b_o, g1, be1, w1, b1, w2, b2, g2, be2):
    f = np.float32
    x = np.asarray(x, f)
    w_qkv = np.asarray(w_qkv, f)
    b_qkv = np.asarray(b_qkv, f)
    w_o = np.asarray(w_o, f)
    w1 = np.asarray(w1, f)
    w2 = np.asarray(w2, f)

    bc = lambda v, n=D: np.ascontiguousarray(
        np.broadcast_to(np.asarray(v, f).reshape(1, n), (P, n)))
    # w1/w2 in streaming-chunk SBUF layout
    w1h = np.ascontiguousarray(
        w1.reshape(DC, P, FC, P).transpose(2, 1, 0, 3)     # [fc, k, dc, m]
        .reshape(FC // 4, 4, P, DC, P).transpose(0, 2, 1, 3, 4)
        .astype(BFNP))
    w2h = np.ascontiguousarray(
        w2.reshape(FC, P, D).reshape(FC // 4, 4, P, D).transpose(0, 2, 1, 3)
        .astype(BFNP))
    shared = {
        "w1": w1h, "w2": w2h,
        "b1": np.ascontiguousarray(np.asarray(b1, f).reshape(FC, P).T),
        "b2b": bc(b2),
        "g1b": bc(g1), "be1b": bc(be1), "g2b": bc(g2), "be2b": bc(be2),
        "vones": np.ones((P, KT), f),
    }
    in_maps = []
    for c in range(8):
        n, r = divmod(c, 4)
        cols = slice(r * 256, (r + 1) * 256)
        xTn = np.ascontiguousarray(
            x[n].T.reshape(DC, P, L).transpose(1, 0, 2).astype(BFNP))
        rows = np.concatenate(
            [np.arange(q * QL + r * P, q * QL + (r + 1) * P)
             for q in range(QD)])
        xqn = np.ascontiguousarray(
            (x[n][rows] + np.asarray(b_o, f).reshape(1, D))
            .reshape(QD, P, D).transpose(1, 0, 2))
        m = dict(shared)
        m["xT"] = xTn
        m["xq"] = xqn
        m["wq"] = np.ascontiguousarray(
            w_qkv[:, :D][:, cols].reshape(DC, P, NP, P)
            .transpose(1, 2, 0, 3).astype(BFNP))
        m["wk"] = np.ascontiguousarray(
            w_qkv[:, D:2 * D][:, cols].reshape(DC, P, NP, P)
            .transpose(1, 2, 0, 3).astype(BFNP))
        m["wv"] = np.ascontiguousarray(
            w_qkv[:, 2 * D:][:, cols].reshape(DC, P, 2 * P)
            .transpose(1, 0, 2).astype(BFNP))
        m["wo"] = np.ascontiguousarray(
            w_o[cols, :].reshape(NP, P, D).transpose(1, 0, 2).astype(BFNP))
        m["bq"] = np.ascontiguousarray(b_qkv[:D][cols].reshape(NP, P).T)
        m["bk"] = np.ascontiguousarray(b_qkv[D:2 * D][cols].reshape(NP, P).T)
        m["bv"] = bc(b_qkv[2 * D:][cols], 2 * P)
        in_maps.append(m)
    return in_maps


def get_nc(affine=True):
    if affine not in _CACHED_NC:
        _CACHED_NC[affine] = _build_nc(affine)
    return _CACHED_NC[affine]


def kernel(**inputs):
    in_maps = make_in_maps(**inputs)
    affine = not (np.all(np.asarray(inputs["g1"]) == 1)
                  and np.all(np.asarray(inputs["be1"]) == 0)
                  and np.all(np.asarray(inputs["g2"]) == 1)
                  and np.all(np.asarray(inputs["be2"]) == 0))
    nc = get_nc(affine)
    # The axon-proxied NRT occasionally reports a transient
    # NRT_EXEC_UNIT_UNRECOVERABLE on a cold first dispatch; a plain retry
    # has always succeeded with bit-identical results, so recover inline.
    last_err = None
    for _ in range(3):
        try:
            res = run_bass_kernel_spmd(nc, in_maps, list(range(8))).results
            break
        except Exception as e:  # noqa: BLE001
            last_err = e
    else:
        raise last_err
    out = np.empty((NB, L, D), np.float32)
    for c in range(8):
        n, r = divmod(c, 4)
        yv = res[c]["y"]
        for q in range(QD):
            out[n, q * QL + r * P:q * QL + (r + 1) * P] = yv[q]
    return out


if __name__ == "__main__":
    rng = np.random.default_rng(0)
    demo = {
        "x": rng.standard_normal((NB, L, D)).astype(np.float32),
        "w_qkv": rng.standard_normal((D, 3 * D)).astype(np.float32) * 0.03,
        "b_qkv": rng.standard_normal(3 * D).astype(np.float32) * 0.03,
        "w_o": rng.standard_normal((D, D)).astype(np.float32) * 0.03,
        "b_o": rng.standard_normal(D).astype(np.float32) * 0.03,
        "g1": np.ones(D, np.float32), "be1": np.zeros(D, np.float32),
        "w1": rng.standard_normal((D, FF)).astype(np.float32) * 0.03,
        "b1": rng.standard_normal(FF).astype(np.float32) * 0.03,
        "w2": rng.standard_normal((FF, D)).astype(np.float32) * 0.015,
        "b2": rng.standard_normal(D).astype(np.float32) * 0.015,
        "g2": np.ones(D, np.float32), "be2": np.zeros(D, np.float32),
    }
    out = kernel(**demo)
    print("kernel output:", out.shape, out.dtype, np.abs(out).mean())


# revision 29
# speedup vs baseline: 1.0280x; 1.0280x over previous
"""Trainium2 Bass kernel for a transformer encoder layer (nn_Encoder).

x:[2,2048,1024] f32, 8 NeuronCores. Hybrid data/tensor parallel: core c
handles batch n=c//4 and head-group r=c%4 (4 of 16 heads). Each core
computes Q/K/V for its 4 heads over all 2048 tokens (no K/V recompute
redundancy), runs attention per 512-query quarter, then a partial output
projection; a per-quarter ReduceScatter over the 4-core group sums the
partials and hands each core 128 tokens per quarter (512 total) for the
LN1+FFN+LN2 tail. The 4 ReduceScatters run on the collective cores,
overlapped with attention of later quarters.

Matmul operands are bf16 (same PE rate as f32r, half the DMA/SBUF);
accumulation and the collective stay f32. LN stats in f32.
"""
import os
import sys

for _p in ("/opt/trn_rl_repo", "/root/.axon_site/_ro/trn_rl_repo"):
    if os.path.isdir(_p) and _p not in sys.path:
        sys.path.insert(0, _p)

import numpy as np
import ml_dtypes
import concourse.bass as bass
import concourse.mybir as mybir
import concourse.tile as tile
from concourse import bacc
from concourse.bass_utils import run_bass_kernel_spmd
from concourse.masks import make_identity

F32 = mybir.dt.float32
BF16 = mybir.dt.bfloat16
AF = mybir.ActivationFunctionType
ALU = mybir.AluOpType
BFNP = ml_dtypes.bfloat16

D = 1024
H = 16
HD = 64
FF = 4096
L = 2048
NB = 2
P = 128
DC = D // P       # 8 chunks of the model dim
KT = L // P       # 16 key tiles
FC = FF // P      # 32 ff chunks
NP = 2            # head pairs per core (4 heads)
QD = 4            # query quarters
QL = L // QD      # 512 queries per quarter
EPS = 1e-5
GROUPS = [[0, 1, 2, 3], [4, 5, 6, 7]]

_CACHED_NC = {}


def _layernorm(nc, pool, dst, src, g_t, be_t, eps_t, zero_t, affine):
    """dst = (src - mean)/sqrt(var + eps) [* g + be], row-wise over 1024.

    var = E[x^2] - mu^2 (safe here: |mu| << rms). One Newton step refines
    the reciprocal sqrt.
    """
    mu = pool.tile([P, 1], F32, tag="ln_mu")
    nc.vector.tensor_reduce(mu[:], src, mybir.AxisListType.X, ALU.add)
    nc.vector.tensor_scalar_mul(mu[:], mu[:], 1.0 / D)
    c = pool.tile([P, D], F32, tag="ln_c")
    ss = pool.tile([P, 1], F32, tag="ln_ss")
    nc.scalar.activation(c[:], src, AF.Square, accum_out=ss[:])
    vv = pool.tile([P, 1], F32, tag="ln_v")
    nc.vector.tensor_scalar(vv[:], ss[:], 1.0 / D, EPS, ALU.mult, ALU.add)
    m2 = pool.tile([P, 1], F32, tag="ln_m2")
    nc.vector.tensor_tensor(m2[:], mu[:], mu[:], ALU.mult)
    nc.vector.tensor_tensor(vv[:], vv[:], m2[:], ALU.subtract)
    s = pool.tile([P, 1], F32, tag="ln_s")
    nc.scalar.activation(s[:], vv[:], AF.Sqrt, bias=zero_t[:])
    r = pool.tile([P, 1], F32, tag="ln_r")
    nc.vector.reciprocal(r[:], s[:])
    t = pool.tile([P, 1], F32, tag="ln_t")
    nc.vector.tensor_tensor(t[:], r[:], r[:], ALU.mult)
    nc.vector.tensor_tensor(t[:], t[:], vv[:], ALU.mult)
    nc.vector.tensor_scalar(t[:], t[:], -0.5, 1.5, ALU.mult, ALU.add)
    nc.vector.tensor_tensor(r[:], r[:], t[:], ALU.mult)
    nc.vector.tensor_scalar(dst, src, mu[:], r[:], ALU.subtract, ALU.mult)
    if affine:
        nc.vector.tensor_tensor(dst, dst, g_t[:], ALU.mult)
        nc.vector.tensor_tensor(dst, dst, be_t[:], ALU.add)


def _build_nc(affine=True):
    nc = bacc.Bacc("TRN2", target_bir_lowering=False, num_devices=8)

    def dparam(name, shape, dt=BF16):
        return nc.dram_tensor(name, shape, dt, kind="ExternalInput")

    xT = dparam("xT", [P, DC, L])           # x[n].T as [p, dc, t] (d=dc*128+p)
    xq = dparam("xq", [P, QD, D])           # owned token tiles, + b_o folded
    wq = dparam("wq", [P, NP, DC, P])       # [dpart, pair, dchunk, qcols]
    wk = dparam("wk", [P, NP, DC, P])
    wv = dparam("wv", [P, DC, 2 * P])       # [dpart, dchunk, vcols(4 heads)]
    wo = dparam("wo", [P, NP, D])           # [hd-part, pair, ocols]
    w1 = dparam("w1", [FC // 4, P, 4, DC, P])  # per-fc4 chunk, SBUF layout
    w2 = dparam("w2", [FC // 4, P, 4, D])
    bq = dparam("bq", [P, NP], F32)
    bk = dparam("bk", [P, NP], F32)
    bv = dparam("bv", [P, 2 * P], F32)      # per-column bias, broadcast rows
    b1 = dparam("b1", [P, FC], F32)
    b2b = dparam("b2b", [P, D], F32)
    g1b = dparam("g1b", [P, D], F32)
    be1b = dparam("be1b", [P, D], F32)
    g2b = dparam("g2b", [P, D], F32)
    be2b = dparam("be2b", [P, D], F32)
    vones = dparam("vones", [P, KT], F32)

    parts = [nc.dram_tensor(f"part{q}", [QL, D], F32) for q in range(QD)]
    reds = [nc.dram_tensor(f"red{q}", [P, D], F32) for q in range(QD)]
    y = nc.dram_tensor("y", [QD, P, D], F32, kind="ExternalOutput")

    with tile.TileContext(nc) as tc:
        with tc.tile_pool(name="pers", bufs=1) as pers, \
             tc.tile_pool(name="wp", bufs=1) as wp, \
             tc.tile_pool(name="stg", bufs=2) as stg, \
             tc.tile_pool(name="atp", bufs=3) as atp, \
             tc.tile_pool(name="atd", bufs=2) as atd, \
             tc.tile_pool(name="hpp", bufs=2) as hpp, \
             tc.tile_pool(name="lnp", bufs=2) as lnp, \
             tc.tile_pool(name="fp", bufs=2) as fp, \
             tc.tile_pool(name="fp2", bufs=2) as fp2, \
             tc.tile_pool(name="t2p", bufs=2) as t2p, \
             tc.tile_pool(name="ps", bufs=1, space="PSUM") as ps:

            # ---- persistent SBUF ----
            xT_t = pers.tile([P, DC, L], BF16, tag="blobA")      # 32KB
            qT_t = pers.tile([P, NP, L], BF16, tag="qT")
            kT_t = pers.tile([P, NP, L], BF16, tag="kT")
            v_aug = pers.tile([P, KT, 4, HD + 1], BF16, tag="vaug")
            outSB = pers.tile([P, NP, QL], BF16, tag="outSB")
            h_t = pers.tile([P, QD, D], F32, tag="h")            # post-LN1
            hb_t = pers.tile([P, QD, D], F32, tag="hb")          # h + b2
            hT_t = pers.tile([P, DC, QD * P], BF16, tag="hT")
            xq_t = pers.tile([P, QD, D], BF16, tag="xq")
            eps_t = pers.tile([P, 1], F32, tag="eps")
            zero_t = pers.tile([P, 1], F32, tag="zero")
            ident = pers.tile([P, P], F32, tag="ident")
            nc.gpsimd.memset(eps_t[:], EPS)
            nc.gpsimd.memset(zero_t[:], 0.0)
            make_identity(nc, ident[:])

            # psum tiles: fixed 8-bank plan, manually assigned per phase
            def pt(tag, name, cols=1024):
                return ps.tile([P, cols], F32, tag=tag, name=name)

            # ---- weight / bias prefetch (scalar queue) ----
            wq_t = wp.tile([P, NP, DC, P], BF16, tag="wq")
            wk_t = wp.tile([P, NP, DC, P], BF16, tag="wk")
            wv_t = wp.tile([P, DC, 2 * P], BF16, tag="wv")
            wo_t = wp.tile([P, NP, D], BF16, tag="wo")
            bq_t = wp.tile([P, NP], F32, tag="bq")
            bk_t = wp.tile([P, NP], F32, tag="bk")
            bv_t = wp.tile([P, 2 * P], F32, tag="bv")
            b1_t = wp.tile([P, FC], F32, tag="b1")
            b2b_t = wp.tile([P, D], F32, tag="b2b")
            ones_t = wp.tile([P, KT], F32, tag="ones")
            # wq pair0 first so the very first Q matmul can start sooner
            nc.scalar.dma_start(wq_t[:, 0], wq[:, 0])
            nc.scalar.dma_start(wq_t[:, 1], wq[:, 1])
            nc.scalar.dma_start(wk_t[:], wk[:])
            nc.scalar.dma_start(wv_t[:], wv[:])
            nc.scalar.dma_start(wo_t[:], wo[:])
            nc.scalar.dma_start(bq_t[:], bq[:])
            nc.scalar.dma_start(bk_t[:], bk[:])
            nc.scalar.dma_start(bv_t[:], bv[:])
            nc.scalar.dma_start(b1_t[:], b1[:])
            nc.scalar.dma_start(b2b_t[:], b2b[:])
            nc.scalar.dma_start(ones_t[:], vones[:])
            if affine:
                g1b_t = wp.tile([P, D], F32, tag="g1b")
                be1b_t = wp.tile([P, D], F32, tag="be1b")
                g2b_t = wp.tile([P, D], F32, tag="g2b")
                be2b_t = wp.tile([P, D], F32, tag="be2b")
                nc.scalar.dma_start(g1b_t[:], g1b[:])
                nc.scalar.dma_start(be1b_t[:], be1b[:])
                nc.scalar.dma_start(g2b_t[:], g2b[:])
                nc.scalar.dma_start(be2b_t[:], be2b[:])
            else:
                g1b_t = be1b_t = g2b_t = be2b_t = None

            # x stream (sync queue), per-dc so Q proj starts on chunk 0
            for dc in range(DC):
                nc.sync.dma_start(xT_t[:, dc, :], xT[:, dc, :])
            nc.sync.dma_start(xq_t[:], xq[:])

            # 8 half-bank accumulation slots for the projection phases.
            # One tile per tag per phase; slots address halves explicitly
            # (repeated same-tag tile creation would WAW-serialize).
            def phase_slots(phase):
                tiles = {t: pt(t, f"{phase}_{t}",
                               cols=512 if t in ("pE", "pF") else 1024)
                         for t in ("pAB", "pCD", "pGH", "pE", "pF")}
                layout = [("pAB", 0), ("pAB", 512), ("pCD", 0), ("pCD", 512),
                          ("pGH", 0), ("pGH", 512), ("pE", 0), ("pF", 0)]
                return [(tiles[t], off) for t, off in layout]

            # ================= Q projection (dc-outer sweeps) ==========
            # dc-outer so the first matmuls ride the xT chunk stream.
            qsl = phase_slots("q")
            for pair in range(NP):
                for dc in range(DC):
                    for qt in range(4):
                        tile_, off = qsl[pair * 4 + qt]
                        nc.tensor.matmul(tile_[:, off:off + 512],
                                         wq_t[:, pair, dc, :],
                                         xT_t[:, dc, qt * 512:(qt + 1) * 512],
                                         start=(dc == 0), stop=(dc == DC - 1))
                for qt in range(4):
                    tile_, off = qsl[pair * 4 + qt]
                    nc.vector.tensor_scalar(
                        qT_t[:, pair, qt * 512:(qt + 1) * 512],
                        tile_[:, off:off + 512],
                        bq_t[:, pair:pair + 1], None, ALU.add)

            # ================= K projection (dc-inner) =================
            ksl = phase_slots("k")
            for pair in range(NP):
                for qt in range(4):
                    i = pair * 4 + qt
                    tile_, off = ksl[i]
                    for dc in range(DC):
                        nc.tensor.matmul(tile_[:, off:off + 512],
                                         wk_t[:, pair, dc, :],
                                         xT_t[:, dc, qt * 512:(qt + 1) * 512],
                                         start=(dc == 0), stop=(dc == DC - 1))
                    nc.vector.tensor_scalar(
                        kT_t[:, pair, qt * 512:(qt + 1) * 512],
                        tile_[:, off:off + 512],
                        bk_t[:, pair:pair + 1], None, ALU.add)

            # ================= V projection ============================
            nc.vector.tensor_copy(
                v_aug[:, :, :, HD],
                ones_t[:, :, None].to_broadcast([P, KT, 4]))
            vsl = phase_slots("v")
            for tt in range(KT):
                tile_, off = vsl[tt % 8]
                for dc in range(DC):
                    nc.tensor.matmul(tile_[:, off:off + 256],
                                     xT_t[:, dc, tt * P:(tt + 1) * P],
                                     wv_t[:, dc, :],
                                     start=(dc == 0), stop=(dc == DC - 1))
                nc.vector.tensor_tensor(
                    v_aug[:, tt, :, 0:HD],
                    tile_[:, off:off + 256].rearrange("p (h c) -> p h c", c=HD),
                    bv_t[:].rearrange("p (h c) -> p h c", c=HD),
                    ALU.add)

            # ====== attention + out-proj partial + RS, per quarter =====
            def post_quarter(qd):
                """RS result -> +x residual -> LN1 (transpose issued apart)."""
                hpre = hpp.tile([P, D], F32, tag="hpre", name=f"hpre_{qd}")
                nc.sync.dma_start(hpre[:], reds[qd][:])
                nc.vector.tensor_tensor(hpre[:], hpre[:], xq_t[:, qd, :],
                                        ALU.add)
                _layernorm(nc, lnp, h_t[:, qd, :], hpre[:], g1b_t, be1b_t,
                           eps_t, zero_t, affine)
                nc.vector.tensor_tensor(hb_t[:, qd, :], h_t[:, qd, :],
                                        b2b_t[:], ALU.add)

            def pe_transpose(qd):
                """h_t[:, qd, :] -> hT_t[:, :, qd*P:...] via 8 PE transposes
                through a pGH psum tile, drained by the Pool engine."""
                trp = pt("pGH", f"tr_{qd}")
                for dc in range(DC):
                    nc.tensor.transpose(trp[:, dc * P:(dc + 1) * P],
                                        h_t[:, qd, dc * P:(dc + 1) * P],
                                        ident[:])
                nc.vector.tensor_copy(
                    hT_t[:, :, qd * P:(qd + 1) * P],
                    trp[:].rearrange("p (c t) -> p c t", t=P))

            for qd in range(QD):
                qcols = slice(qd * QL, (qd + 1) * QL)
                # ---- attention for 2 pairs ----
                stprot = ["pAB", "pCD"]
                sti = 0
                for pair in range(NP):
                    pvs = [pt("pE", f"pv_{qd}_{pair}_0", cols=512),
                           pt("pF", f"pv_{qd}_{pair}_1", cols=512)]
                    for ktp in range(KT // 2):
                        # S for both heads first, then both PVs; the stp psum
                        # rotates over 3 double-bank slots so the PE can run
                        # several S groups ahead of the ACT exp stream.
                        ptts = []
                        for h2 in range(2):
                            rows = slice(h2 * HD, h2 * HD + HD)
                            stp = pt(stprot[sti % 2],
                                     f"st_{qd}_{pair}_{ktp}_{h2}")
                            sti += 1
                            for j in range(2):
                                kt = 2 * ktp + j
                                nc.tensor.matmul(
                                    stp[:, j * 512:(j + 1) * 512],
                                    kT_t[rows, pair, kt * P:(kt + 1) * P],
                                    qT_t[rows, pair, qcols],
                                    start=True, stop=True)
                            ptt = atp.tile([P, 2, 512], BF16, tag="pt",
                                           name=f"pt_{qd}_{pair}_{ktp}_{h2}")
                            nc.scalar.activation(
                                ptt[:],
                                stp[:].rearrange("p (c n) -> p c n", n=512),
                                AF.Exp, scale=0.125)
                            ptts.append(ptt)
                        for h2 in range(2):
                            for j in range(2):
                                kt = 2 * ktp + j
                                nc.tensor.matmul(
                                    pvs[h2][:HD + 1, :],
                                    v_aug[:, kt, 2 * pair + h2, :],
                                    ptts[h2][:, j, :],
                                    start=(kt == 0), stop=(kt == KT - 1))
                    for h2 in range(2):
                        rows = slice(h2 * HD, h2 * HD + HD)
                        den = atd.tile([1, QL], F32, tag="den",
                                       name=f"den_{qd}_{pair}_{h2}")
                        nc.vector.reciprocal(den[:], pvs[h2][HD:HD + 1, :])
                        denb = atd.tile([HD, QL], F32, tag="denb",
                                        name=f"denb_{qd}_{pair}_{h2}")
                        nc.gpsimd.partition_broadcast(denb[:], den[:])
                        nc.vector.tensor_tensor(outSB[rows, pair, :],
                                                pvs[h2][:HD, :], denb[:],
                                                ALU.mult)

                # ---- out-proj partial: [512 q, 1024], psum->SBUF->DRAM ----
                poslots = ["pGH", "pAB", "pCD", "pGH"]
                for tt4 in range(4):
                    po = pt(poslots[tt4], f"po_{qd}_{tt4}")
                    for pair in range(NP):
                        for oc in range(2):
                            nc.tensor.matmul(
                                po[:, oc * 512:(oc + 1) * 512],
                                outSB[:, pair, tt4 * P:(tt4 + 1) * P],
                                wo_t[:, pair, oc * 512:(oc + 1) * 512],
                                start=(pair == 0), stop=(pair == NP - 1))
                    pst = stg.tile([P, D], F32, tag="pstg",
                                   name=f"pstg_{qd}_{tt4}")
                    # drain on both DVE and ACT: ACT has a lull at quarter
                    # end while the next quarter's S matmuls prime the exps
                    if tt4 % 2 == 0:
                        nc.vector.tensor_copy(pst[:], po[:])
                    else:
                        nc.scalar.activation(pst[:], po[:], AF.Copy)
                    nc.sync.dma_start(
                        parts[qd].rearrange("(a p) d -> p a d", p=P)[:, tt4, :],
                        pst[:])
                nc.gpsimd.collective_compute(
                    "ReduceScatter", ALU.add, replica_groups=GROUPS,
                    ins=[parts[qd][:].opt()], outs=[reds[qd][:].opt()])
            # posts AFTER the full attention loop, with scheduler hints so
            # the greedy per-engine scheduler does not slot their long
            # collective-gated waits into the attention instruction streams
            # (a wait at a queue head blocks everything behind it).
            for qd in range(QD):
                with tc.tile_wait_until(0.160 + 0.045 * qd):
                    post_quarter(qd)
            for qd in range(QD - 1):
                pe_transpose(qd)

            # ================= FFN =====================================
            # Four phases: fc1(c1) -> fc2(c1) -> fc1(c2) -> fc2(c2).
            # fc2(c1) only needs ff1 for chunk-1 tokens, so it covers the
            # RS_q3 -> LN1_q3 -> transpose latency that gates fc1(c2).
            # w1 and w2 each stream twice (bf16, ~270 GB/s bursts).
            ff1 = pers.tile([P, FC, QD * P], BF16, tag="blobA")

            def fc1_chunk(ch):
                tcols = slice(ch * 256, (ch + 1) * 256)
                f1AB = pt("pAB", f"f1AB_{ch}")
                f1CD = pt("pCD", f"f1CD_{ch}")
                f1slots = [(f1AB, 0), (f1AB, 512), (f1CD, 0), (f1CD, 512)]
                for fc4 in range(FC // 4):
                    w1_t = fp.tile([P, 4, DC, P], BF16, tag="w1s")
                    nc.sync.dma_start(w1_t[:], w1[fc4])
                    for f in range(4):
                        fc = 4 * fc4 + f
                        tile_, sl = f1slots[f]
                        for dc in range(DC):
                            nc.tensor.matmul(
                                tile_[:, sl:sl + 256],
                                w1_t[:, f, dc, :], hT_t[:, dc, tcols],
                                start=(dc == 0), stop=(dc == DC - 1))
                        nc.vector.tensor_scalar(
                            ff1[:, fc, tcols], tile_[:, sl:sl + 256],
                            b1_t[:, fc:fc + 1], 0.0, ALU.add, ALU.max)

            def fc2_chunk(ch):
                tts = (2 * ch, 2 * ch + 1)
                tGH = pt("pGH", f"f2GH_{ch}")
                tE = pt("pE", f"f2E_{ch}", cols=512)
                tF = pt("pF", f"f2F_{ch}", cols=512)
                dsts = {(tts[0], 0): tGH[:, 0:512], (tts[0], 1): tGH[:, 512:1024],
                        (tts[1], 0): tE[:, :], (tts[1], 1): tF[:, :]}
                for fc4 in range(FC // 4):
                    w2_t = fp2.tile([P, 4, D], BF16, tag="w2s")
                    nc.sync.dma_start(w2_t[:], w2[fc4])
                    for f in range(4):
                        fc = 4 * fc4 + f
                        st = (fc == 0)
                        sp = (fc == FC - 1)
                        for tt in tts:
                            for oc in range(2):
                                nc.tensor.matmul(
                                    dsts[(tt, oc)],
                                    ff1[:, fc, tt * P:(tt + 1) * P],
                                    w2_t[:, f, oc * 512:(oc + 1) * 512],
                                    start=st, stop=sp)
                for i, tt in enumerate(tts):
                    t2 = t2p.tile([P, D], F32, tag="t2", name=f"t2_{tt}")
                    if i == 0:
                        nc.vector.tensor_tensor(t2[:], tGH[:],
                                                hb_t[:, tt, :], ALU.add)
                    else:
                        nc.vector.tensor_tensor(t2[:, 0:512], tE[:],
                                                hb_t[:, tt, 0:512], ALU.add)
                        nc.vector.tensor_tensor(t2[:, 512:1024], tF[:],
                                                hb_t[:, tt, 512:1024], ALU.add)
                    _layernorm(nc, lnp, t2[:], t2[:], g2b_t, be2b_t, eps_t,
                               zero_t, affine)
                    nc.sync.dma_start(y[tt], t2[:])

            fc1_chunk(0)
            fc2_chunk(0)
            pe_transpose(QD - 1)
            fc1_chunk(1)
            fc2_chunk(1)

    nc.compile()
    return nc


def make_in_maps(x, w_qkv, b_qkv, w_o, b_o, g1, be1, w1, b1, w2, b2, g2, be2):
    f = np.float32
    x = np.asarray(x, f)
    w_qkv = np.asarray(w_qkv, f)
    b_qkv = np.asarray(b_qkv, f)
    w_o = np.asarray(w_o, f)
    w1 = np.asarray(w1, f)
    w2 = np.asarray(w2, f)

    bc = lambda v, n=D: np.ascontiguousarray(
        np.broadcast_to(np.asarray(v, f).reshape(1, n), (P, n)))
    # w1/w2 in streaming-chunk SBUF layout
    w1h = np.ascontiguousarray(
        w1.reshape(DC, P, FC, P).transpose(2, 1, 0, 3)     # [fc, k, dc, m]
        .reshape(FC // 4, 4, P, DC, P).transpose(0, 2, 1, 3, 4)
        .astype(BFNP))
    w2h = np.ascontiguousarray(
        w2.reshape(FC, P, D).reshape(FC // 4, 4, P, D).transpose(0, 2, 1, 3)
        .astype(BFNP))
    shared = {
        "w1": w1h, "w2": w2h,
        "b1": np.ascontiguousarray(np.asarray(b1, f).reshape(FC, P).T),
        "b2b": bc(b2),
        "g1b": bc(g1), "be1b": bc(be1), "g2b": bc(g2), "be2b": bc(be2),
        "vones": np.ones((P, KT), f),
    }
    in_maps = []
    for c in range(8):
        n, r = divmod(c, 4)
        cols = slice(r * 256, (r + 1) * 256)
        xTn = np.ascontiguousarray(
            x[n].T.reshape(DC, P, L).transpose(1, 0, 2).astype(BFNP))
        rows = np.concatenate(
            [np.arange(q * QL + r * P, q * QL + (r + 1) * P)
             for q in range(QD)])
        xqn = np.ascontiguousarray(
            (x[n][rows] + np.asarray(b_o, f).reshape(1, D))
            .reshape(QD, P, D).transpose(1, 0, 2).astype(BFNP))
        m = dict(shared)
        m["xT"] = xTn
        m["xq"] = xqn
        m["wq"] = np.ascontiguousarray(
            w_qkv[:, :D][:, cols].reshape(DC, P, NP, P)
            .transpose(1, 2, 0, 3).astype(BFNP))
        m["wk"] = np.ascontiguousarray(
            w_qkv[:, D:2 * D][:, cols].reshape(DC, P, NP, P)
            .transpose(1, 2, 0, 3).astype(BFNP))
        m["wv"] = np.ascontiguousarray(
            w_qkv[:, 2 * D:][:, cols].reshape(DC, P, 2 * P)
            .transpose(1, 0, 2).astype(BFNP))
        m["wo"] = np.ascontiguousarray(
            w_o[cols, :].reshape(NP, P, D).transpose(1, 0, 2).astype(BFNP))
        m["bq"] = np.ascontiguousarray(b_qkv[:D][cols].reshape(NP, P).T)
        m["bk"] = np.ascontiguousarray(b_qkv[D:2 * D][cols].reshape(NP, P).T)
        m["bv"] = bc(b_qkv[2 * D:][cols], 2 * P)
        in_maps.append(m)
    return in_maps


def get_nc(affine=True):
    if affine not in _CACHED_NC:
        _CACHED_NC[affine] = _build_nc(affine)
    return _CACHED_NC[affine]


def kernel(**inputs):
    in_maps = make_in_maps(**inputs)
    affine = not (np.all(np.asarray(inputs["g1"]) == 1)
                  and np.all(np.asarray(inputs["be1"]) == 0)
                  and np.all(np.asarray(inputs["g2"]) == 1)
                  and np.all(np.asarray(inputs["be2"]) == 0))
    nc = get_nc(affine)
    # The axon-proxied NRT occasionally reports a transient
    # NRT_EXEC_UNIT_UNRECOVERABLE on a cold first dispatch; a plain retry
    # has always succeeded with bit-identical results, so recover inline.
    last_err = None
    for _ in range(3):
        try:
            res = run_bass_kernel_spmd(nc, in_maps, list(range(8))).results
            break
        except Exception as e:  # noqa: BLE001
            last_err = e
    else:
        raise last_err
    out = np.empty((NB, L, D), np.float32)
    for c in range(8):
        n, r = divmod(c, 4)
        yv = res[c]["y"]
        for q in range(QD):
            out[n, q * QL + r * P:q * QL + (r + 1) * P] = yv[q]
    return out


if __name__ == "__main__":
    rng = np.random.default_rng(0)
    demo = {
        "x": rng.standard_normal((NB, L, D)).astype(np.float32),
        "w_qkv": rng.standard_normal((D, 3 * D)).astype(np.float32) * 0.03,
        "b_qkv": rng.standard_normal(3 * D).astype(np.float32) * 0.03,
        "w_o": rng.standard_normal((D, D)).astype(np.float32) * 0.03,
        "b_o": rng.standard_normal(D).astype(np.float32) * 0.03,
        "g1": np.ones(D, np.float32), "be1": np.zeros(D, np.float32),
        "w1": rng.standard_normal((D, FF)).astype(np.float32) * 0.03,
        "b1": rng.standard_normal(FF).astype(np.float32) * 0.03,
        "w2": rng.standard_normal((FF, D)).astype(np.float32) * 0.015,
        "b2": rng.standard_normal(D).astype(np.float32) * 0.015,
        "g2": np.ones(D, np.float32), "be2": np.zeros(D, np.float32),
    }
    out = kernel(**demo)
    print("kernel output:", out.shape, out.dtype, np.abs(out).mean())


# revision 33
# speedup vs baseline: 1.0458x; 1.0173x over previous
"""Trainium2 Bass kernel for a transformer encoder layer (nn_Encoder).

x:[2,2048,1024] f32, 8 NeuronCores. Hybrid data/tensor parallel: core c
handles batch n=c//4 and head-group r=c%4 (4 of 16 heads). Each core
computes Q/K/V for its 4 heads over all 2048 tokens (no K/V recompute
redundancy), runs attention per 512-query quarter, then a partial output
projection; a per-quarter ReduceScatter over the 4-core group sums the
partials and hands each core 128 tokens per quarter (512 total) for the
LN1+FFN+LN2 tail. The 4 ReduceScatters run on the collective cores,
overlapped with attention of later quarters.

Matmul operands are bf16 (same PE rate as f32r, half the DMA/SBUF);
accumulation and the collective stay f32. LN stats in f32.
"""
import os
import sys

for _p in ("/opt/trn_rl_repo", "/root/.axon_site/_ro/trn_rl_repo"):
    if os.path.isdir(_p) and _p not in sys.path:
        sys.path.insert(0, _p)

import numpy as np
import ml_dtypes
import concourse.bass as bass
import concourse.mybir as mybir
import concourse.tile as tile
from concourse import bacc
from concourse.bass_utils import run_bass_kernel_spmd
from concourse.masks import make_identity

F32 = mybir.dt.float32
BF16 = mybir.dt.bfloat16
AF = mybir.ActivationFunctionType
ALU = mybir.AluOpType
BFNP = ml_dtypes.bfloat16

D = 1024
H = 16
HD = 64
FF = 4096
L = 2048
NB = 2
P = 128
DC = D // P       # 8 chunks of the model dim
KT = L // P       # 16 key tiles
FC = FF // P      # 32 ff chunks
NP = 2            # head pairs per core (4 heads)
QD = 4            # query quarters
QL = L // QD      # 512 queries per quarter
EPS = 1e-5
GROUPS = [[0, 1, 2, 3], [4, 5, 6, 7]]

_CACHED_NC = {}


def _layernorm(nc, pool, dst, src, g_t, be_t, eps_t, zero_t, affine):
    """dst = (src - mean)/sqrt(var + eps) [* g + be], row-wise over 1024.

    var = E[x^2] - mu^2 (safe here: |mu| << rms). One Newton step refines
    the reciprocal sqrt.
    """
    mu = pool.tile([P, 1], F32, tag="ln_mu")
    nc.vector.tensor_reduce(mu[:], src, mybir.AxisListType.X, ALU.add)
    nc.vector.tensor_scalar_mul(mu[:], mu[:], 1.0 / D)
    c = pool.tile([P, D], F32, tag="ln_c")
    ss = pool.tile([P, 1], F32, tag="ln_ss")
    nc.scalar.activation(c[:], src, AF.Square, accum_out=ss[:])
    vv = pool.tile([P, 1], F32, tag="ln_v")
    nc.vector.tensor_scalar(vv[:], ss[:], 1.0 / D, EPS, ALU.mult, ALU.add)
    m2 = pool.tile([P, 1], F32, tag="ln_m2")
    nc.vector.tensor_tensor(m2[:], mu[:], mu[:], ALU.mult)
    nc.vector.tensor_tensor(vv[:], vv[:], m2[:], ALU.subtract)
    s = pool.tile([P, 1], F32, tag="ln_s")
    nc.scalar.activation(s[:], vv[:], AF.Sqrt, bias=zero_t[:])
    r = pool.tile([P, 1], F32, tag="ln_r")
    nc.vector.reciprocal(r[:], s[:])
    t = pool.tile([P, 1], F32, tag="ln_t")
    nc.vector.tensor_tensor(t[:], r[:], r[:], ALU.mult)
    nc.vector.tensor_tensor(t[:], t[:], vv[:], ALU.mult)
    nc.vector.tensor_scalar(t[:], t[:], -0.5, 1.5, ALU.mult, ALU.add)
    nc.vector.tensor_tensor(r[:], r[:], t[:], ALU.mult)
    nc.vector.tensor_scalar(dst, src, mu[:], r[:], ALU.subtract, ALU.mult)
    if affine:
        nc.vector.tensor_tensor(dst, dst, g_t[:], ALU.mult)
        nc.vector.tensor_tensor(dst, dst, be_t[:], ALU.add)


def _build_nc(affine=True):
    nc = bacc.Bacc("TRN2", target_bir_lowering=False, num_devices=8)

    def dparam(name, shape, dt=BF16):
        return nc.dram_tensor(name, shape, dt, kind="ExternalInput")

    xT = dparam("xT", [P, DC, L])           # x[n].T as [p, dc, t] (d=dc*128+p)
    xq = dparam("xq", [P, QD, D])           # owned token tiles, + b_o folded
    wq = dparam("wq", [P, NP, DC, P])       # [dpart, pair, dchunk, qcols]
    wk = dparam("wk", [P, NP, DC, P])
    wv = dparam("wv", [P, DC, 2 * P])       # [dpart, dchunk, vcols(4 heads)]
    wo = dparam("wo", [P, NP, D])           # [hd-part, pair, ocols]
    w1 = dparam("w1", [FC // 4, P, 4, DC, P])  # per-fc4 chunk, SBUF layout
    w2 = dparam("w2", [FC // 4, P, 4, D])
    bq = dparam("bq", [P, NP], F32)
    bk = dparam("bk", [P, NP], F32)
    bv = dparam("bv", [P, 2 * P], F32)      # per-column bias, broadcast rows
    b1 = dparam("b1", [P, FC], F32)
    b2b = dparam("b2b", [P, D], F32)
    g1b = dparam("g1b", [P, D], F32)
    be1b = dparam("be1b", [P, D], F32)
    g2b = dparam("g2b", [P, D], F32)
    be2b = dparam("be2b", [P, D], F32)
    vones = dparam("vones", [P, KT], F32)

    parts = [nc.dram_tensor(f"part{q}", [QL, D], F32) for q in range(QD)]
    reds = [nc.dram_tensor(f"red{q}", [P, D], F32) for q in range(QD)]
    y = nc.dram_tensor("y", [QD, P, D], F32, kind="ExternalOutput")

    with tile.TileContext(nc) as tc:
        with tc.tile_pool(name="pers", bufs=1) as pers, \
             tc.tile_pool(name="wp", bufs=1) as wp, \
             tc.tile_pool(name="stg", bufs=2) as stg, \
             tc.tile_pool(name="atp", bufs=3) as atp, \
             tc.tile_pool(name="atd", bufs=2) as atd, \
             tc.tile_pool(name="hpp", bufs=2) as hpp, \
             tc.tile_pool(name="lnp", bufs=2) as lnp, \
             tc.tile_pool(name="fp", bufs=2) as fp, \
             tc.tile_pool(name="fp2", bufs=2) as fp2, \
             tc.tile_pool(name="t2p", bufs=2) as t2p, \
             tc.tile_pool(name="ps", bufs=1, space="PSUM") as ps:

            # ---- persistent SBUF ----
            xT_t = pers.tile([P, DC, L], BF16, tag="blobA")      # 32KB
            qT_t = pers.tile([P, NP, L], BF16, tag="qT")
            kT_t = pers.tile([P, NP, L], BF16, tag="kT")
            v_aug = pers.tile([P, KT, 4, HD + 1], BF16, tag="vaug")
            outSB = pers.tile([P, NP, QL], BF16, tag="outSB")
            h_t = pers.tile([P, QD, D], F32, tag="h")            # post-LN1
            hb_t = pers.tile([P, QD, D], F32, tag="hb")          # h + b2
            hT_t = pers.tile([P, DC, QD * P], BF16, tag="hT")
            xq_t = pers.tile([P, QD, D], BF16, tag="xq")
            eps_t = pers.tile([P, 1], F32, tag="eps")
            zero_t = pers.tile([P, 1], F32, tag="zero")
            ident = pers.tile([P, P], F32, tag="ident")
            nc.gpsimd.memset(eps_t[:], EPS)
            nc.gpsimd.memset(zero_t[:], 0.0)
            make_identity(nc, ident[:])

            # psum tiles: fixed 8-bank plan, manually assigned per phase
            def pt(tag, name, cols=1024):
                return ps.tile([P, cols], F32, tag=tag, name=name)

            # ---- weight / bias prefetch (scalar queue) ----
            wq_t = wp.tile([P, NP, DC, P], BF16, tag="wq")
            wk_t = wp.tile([P, NP, DC, P], BF16, tag="wk")
            wv_t = wp.tile([P, DC, 2 * P], BF16, tag="wv")
            wo_t = wp.tile([P, NP, D], BF16, tag="wo")
            bq_t = wp.tile([P, NP], F32, tag="bq")
            bk_t = wp.tile([P, NP], F32, tag="bk")
            bv_t = wp.tile([P, 2 * P], F32, tag="bv")
            b1_t = wp.tile([P, FC], F32, tag="b1")
            b2b_t = wp.tile([P, D], F32, tag="b2b")
            ones_t = wp.tile([P, KT], F32, tag="ones")
            # wq pair0 first so the very first Q matmul can start sooner
            nc.scalar.dma_start(wq_t[:, 0], wq[:, 0])
            nc.scalar.dma_start(wq_t[:, 1], wq[:, 1])
            nc.scalar.dma_start(wk_t[:], wk[:])
            nc.scalar.dma_start(wv_t[:], wv[:])
            nc.scalar.dma_start(wo_t[:], wo[:])
            nc.scalar.dma_start(bq_t[:], bq[:])
            nc.scalar.dma_start(bk_t[:], bk[:])
            nc.scalar.dma_start(bv_t[:], bv[:])
            nc.scalar.dma_start(b1_t[:], b1[:])
            nc.scalar.dma_start(b2b_t[:], b2b[:])
            nc.scalar.dma_start(ones_t[:], vones[:])
            if affine:
                g1b_t = wp.tile([P, D], F32, tag="g1b")
                be1b_t = wp.tile([P, D], F32, tag="be1b")
                g2b_t = wp.tile([P, D], F32, tag="g2b")
                be2b_t = wp.tile([P, D], F32, tag="be2b")
                nc.scalar.dma_start(g1b_t[:], g1b[:])
                nc.scalar.dma_start(be1b_t[:], be1b[:])
                nc.scalar.dma_start(g2b_t[:], g2b[:])
                nc.scalar.dma_start(be2b_t[:], be2b[:])
            else:
                g1b_t = be1b_t = g2b_t = be2b_t = None

            # x stream (sync queue), per-dc so Q proj starts on chunk 0
            for dc in range(DC):
                nc.sync.dma_start(xT_t[:, dc, :], xT[:, dc, :])
            nc.sync.dma_start(xq_t[:], xq[:])

            # 8 half-bank accumulation slots for the projection phases.
            # One tile per tag per phase; slots address halves explicitly
            # (repeated same-tag tile creation would WAW-serialize).
            def phase_slots(phase):
                tiles = {t: pt(t, f"{phase}_{t}",
                               cols=512 if t in ("pE", "pF") else 1024)
                         for t in ("pAB", "pCD", "pGH", "pE", "pF")}
                layout = [("pAB", 0), ("pAB", 512), ("pCD", 0), ("pCD", 512),
                          ("pGH", 0), ("pGH", 512), ("pE", 0), ("pF", 0)]
                return [(tiles[t], off) for t, off in layout]

            # ================= Q projection (dc-outer sweeps) ==========
            # dc-outer so the first matmuls ride the xT chunk stream.
            qsl = phase_slots("q")
            for pair in range(NP):
                for dc in range(DC):
                    for qt in range(4):
                        tile_, off = qsl[pair * 4 + qt]
                        nc.tensor.matmul(tile_[:, off:off + 512],
                                         wq_t[:, pair, dc, :],
                                         xT_t[:, dc, qt * 512:(qt + 1) * 512],
                                         start=(dc == 0), stop=(dc == DC - 1))
                for qt in range(4):
                    tile_, off = qsl[pair * 4 + qt]
                    nc.vector.tensor_scalar(
                        qT_t[:, pair, qt * 512:(qt + 1) * 512],
                        tile_[:, off:off + 512],
                        bq_t[:, pair:pair + 1], None, ALU.add)

            # ================= K projection (dc-inner) =================
            ksl = phase_slots("k")
            for pair in range(NP):
                for qt in range(4):
                    i = pair * 4 + qt
                    tile_, off = ksl[i]
                    for dc in range(DC):
                        nc.tensor.matmul(tile_[:, off:off + 512],
                                         wk_t[:, pair, dc, :],
                                         xT_t[:, dc, qt * 512:(qt + 1) * 512],
                                         start=(dc == 0), stop=(dc == DC - 1))
                    nc.vector.tensor_scalar(
                        kT_t[:, pair, qt * 512:(qt + 1) * 512],
                        tile_[:, off:off + 512],
                        bk_t[:, pair:pair + 1], None, ALU.add)

            # ================= V projection ============================
            nc.vector.tensor_copy(
                v_aug[:, :, :, HD],
                ones_t[:, :, None].to_broadcast([P, KT, 4]))
            vsl = phase_slots("v")
            for tt in range(KT):
                tile_, off = vsl[tt % 8]
                for dc in range(DC):
                    nc.tensor.matmul(tile_[:, off:off + 256],
                                     xT_t[:, dc, tt * P:(tt + 1) * P],
                                     wv_t[:, dc, :],
                                     start=(dc == 0), stop=(dc == DC - 1))
                nc.vector.tensor_tensor(
                    v_aug[:, tt, :, 0:HD],
                    tile_[:, off:off + 256].rearrange("p (h c) -> p h c", c=HD),
                    bv_t[:].rearrange("p (h c) -> p h c", c=HD),
                    ALU.add)

            # ====== attention + out-proj partial + RS, per quarter =====
            def post_quarter(qd):
                """RS result -> +x residual -> LN1 (transpose issued apart)."""
                hpre = hpp.tile([P, D], F32, tag="hpre", name=f"hpre_{qd}")
                nc.sync.dma_start(hpre[:], reds[qd][:])
                nc.vector.tensor_tensor(hpre[:], hpre[:], xq_t[:, qd, :],
                                        ALU.add)
                _layernorm(nc, lnp, h_t[:, qd, :], hpre[:], g1b_t, be1b_t,
                           eps_t, zero_t, affine)
                nc.vector.tensor_tensor(hb_t[:, qd, :], h_t[:, qd, :],
                                        b2b_t[:], ALU.add)

            def pe_transpose(qd):
                """h_t[:, qd, :] -> hT_t[:, :, qd*P:...] via 8 PE transposes
                through a pGH psum tile, drained by the Pool engine."""
                trp = pt("pGH", f"tr_{qd}")
                for dc in range(DC):
                    nc.tensor.transpose(trp[:, dc * P:(dc + 1) * P],
                                        h_t[:, qd, dc * P:(dc + 1) * P],
                                        ident[:])
                nc.vector.tensor_copy(
                    hT_t[:, :, qd * P:(qd + 1) * P],
                    trp[:].rearrange("p (c t) -> p c t", t=P))

            for qd in range(QD):
                qcols = slice(qd * QL, (qd + 1) * QL)
                # ---- attention for 2 pairs ----
                stprot = ["pAB", "pCD"]
                sti = 0
                for pair in range(NP):
                    pvs = [pt("pE", f"pv_{qd}_{pair}_0", cols=512),
                           pt("pF", f"pv_{qd}_{pair}_1", cols=512)]
                    for ktp in range(KT // 2):
                        # S for both heads first, then both PVs; the stp psum
                        # rotates over 3 double-bank slots so the PE can run
                        # several S groups ahead of the ACT exp stream.
                        ptts = []
                        for h2 in range(2):
                            rows = slice(h2 * HD, h2 * HD + HD)
                            stp = pt(stprot[sti % 2],
                                     f"st_{qd}_{pair}_{ktp}_{h2}")
                            sti += 1
                            for j in range(2):
                                kt = 2 * ktp + j
                                nc.tensor.matmul(
                                    stp[:, j * 512:(j + 1) * 512],
                                    kT_t[rows, pair, kt * P:(kt + 1) * P],
                                    qT_t[rows, pair, qcols],
                                    start=True, stop=True)
                            ptt = atp.tile([P, 2, 512], BF16, tag="pt",
                                           name=f"pt_{qd}_{pair}_{ktp}_{h2}")
                            nc.scalar.activation(
                                ptt[:],
                                stp[:].rearrange("p (c n) -> p c n", n=512),
                                AF.Exp, scale=0.125)
                            ptts.append(ptt)
                        for h2 in range(2):
                            for j in range(2):
                                kt = 2 * ktp + j
                                nc.tensor.matmul(
                                    pvs[h2][:HD + 1, :],
                                    v_aug[:, kt, 2 * pair + h2, :],
                                    ptts[h2][:, j, :],
                                    start=(kt == 0), stop=(kt == KT - 1))
                    for h2 in range(2):
                        rows = slice(h2 * HD, h2 * HD + HD)
                        den = atd.tile([1, QL], F32, tag="den",
                                       name=f"den_{qd}_{pair}_{h2}")
                        nc.vector.reciprocal(den[:], pvs[h2][HD:HD + 1, :])
                        denb = atd.tile([HD, QL], F32, tag="denb",
                                        name=f"denb_{qd}_{pair}_{h2}")
                        nc.gpsimd.partition_broadcast(denb[:], den[:])
                        nc.vector.tensor_tensor(outSB[rows, pair, :],
                                                pvs[h2][:HD, :], denb[:],
                                                ALU.mult)

                # ---- out-proj partial: [512 q, 1024], psum->SBUF->DRAM ----
                poslots = ["pGH", "pAB", "pCD", "pGH"]
                for tt4 in range(4):
                    po = pt(poslots[tt4], f"po_{qd}_{tt4}")
                    for pair in range(NP):
                        for oc in range(2):
                            nc.tensor.matmul(
                                po[:, oc * 512:(oc + 1) * 512],
                                outSB[:, pair, tt4 * P:(tt4 + 1) * P],
                                wo_t[:, pair, oc * 512:(oc + 1) * 512],
                                start=(pair == 0), stop=(pair == NP - 1))
                    pst = stg.tile([P, D], F32, tag="pstg",
                                   name=f"pstg_{qd}_{tt4}")
                    # drain on both DVE and ACT: ACT has a lull at quarter
                    # end while the next quarter's S matmuls prime the exps
                    if tt4 % 2 == 0:
                        nc.vector.tensor_copy(pst[:], po[:])
                    else:
                        nc.scalar.activation(pst[:], po[:], AF.Copy)
                    nc.sync.dma_start(
                        parts[qd].rearrange("(a p) d -> p a d", p=P)[:, tt4, :],
                        pst[:])
                nc.gpsimd.collective_compute(
                    "ReduceScatter", ALU.add, replica_groups=GROUPS,
                    ins=[parts[qd][:].opt()], outs=[reds[qd][:].opt()])
            # posts AFTER the full attention loop, with scheduler hints so
            # the greedy per-engine scheduler does not slot their long
            # collective-gated waits into the attention instruction streams
            # (a wait at a queue head blocks everything behind it).
            for qd in range(QD):
                with tc.tile_wait_until((0.138, 0.185, 0.235, 0.285)[qd]):
                    post_quarter(qd)
            for qd in range(QD - 1):
                pe_transpose(qd)

            # ================= FFN =====================================
            # Four phases: fc1(c1) -> fc2(c1) -> fc1(c2) -> fc2(c2).
            # fc2(c1) only needs ff1 for chunk-1 tokens, so it covers the
            # RS_q3 -> LN1_q3 -> transpose latency that gates fc1(c2).
            # w1 and w2 each stream twice (bf16, ~270 GB/s bursts).
            ff1 = pers.tile([P, FC, QD * P], BF16, tag="blobA")

            def fc1_chunk(ch):
                tcols = slice(ch * 256, (ch + 1) * 256)
                f1AB = pt("pAB", f"f1AB_{ch}")
                f1CD = pt("pCD", f"f1CD_{ch}")
                f1slots = [(f1AB, 0), (f1AB, 512), (f1CD, 0), (f1CD, 512)]
                for fc4 in range(FC // 4):
                    w1_t = fp.tile([P, 4, DC, P], BF16, tag="w1s")
                    nc.sync.dma_start(w1_t[:], w1[fc4])
                    for f in range(4):
                        fc = 4 * fc4 + f
                        tile_, sl = f1slots[f]
                        for dc in range(DC):
                            nc.tensor.matmul(
                                tile_[:, sl:sl + 256],
                                w1_t[:, f, dc, :], hT_t[:, dc, tcols],
                                start=(dc == 0), stop=(dc == DC - 1))
                        nc.vector.tensor_scalar(
                            ff1[:, fc, tcols], tile_[:, sl:sl + 256],
                            b1_t[:, fc:fc + 1], 0.0, ALU.add, ALU.max)

            def fc2_chunk(ch):
                tts = (2 * ch, 2 * ch + 1)
                tGH = pt("pGH", f"f2GH_{ch}")
                tE = pt("pE", f"f2E_{ch}", cols=512)
                tF = pt("pF", f"f2F_{ch}", cols=512)
                dsts = {(tts[0], 0): tGH[:, 0:512], (tts[0], 1): tGH[:, 512:1024],
                        (tts[1], 0): tE[:, :], (tts[1], 1): tF[:, :]}
                for fc4 in range(FC // 4):
                    w2_t = fp2.tile([P, 4, D], BF16, tag="w2s")
                    nc.sync.dma_start(w2_t[:], w2[fc4])
                    for f in range(4):
                        fc = 4 * fc4 + f
                        st = (fc == 0)
                        sp = (fc == FC - 1)
                        for tt in tts:
                            for oc in range(2):
                                nc.tensor.matmul(
                                    dsts[(tt, oc)],
                                    ff1[:, fc, tt * P:(tt + 1) * P],
                                    w2_t[:, f, oc * 512:(oc + 1) * 512],
                                    start=st, stop=sp)
                for i, tt in enumerate(tts):
                    t2 = t2p.tile([P, D], F32, tag="t2", name=f"t2_{tt}")
                    if i == 0:
                        nc.vector.tensor_tensor(t2[:], tGH[:],
                                                hb_t[:, tt, :], ALU.add)
                    else:
                        nc.vector.tensor_tensor(t2[:, 0:512], tE[:],
                                                hb_t[:, tt, 0:512], ALU.add)
                        nc.vector.tensor_tensor(t2[:, 512:1024], tF[:],
                                                hb_t[:, tt, 512:1024], ALU.add)
                    _layernorm(nc, lnp, t2[:], t2[:], g2b_t, be2b_t, eps_t,
                               zero_t, affine)
                    nc.sync.dma_start(y[tt], t2[:])

            fc1_chunk(0)
            fc2_chunk(0)
            pe_transpose(QD - 1)
            fc1_chunk(1)
            fc2_chunk(1)

    nc.compile()
    return nc


def make_in_maps(x, w_qkv, b_qkv, w_o, b_o, g1, be1, w1, b1, w2, b2, g2, be2):
    f = np.float32
    x = np.asarray(x, f)
    w_qkv = np.asarray(w_qkv, f)
    b_qkv = np.asarray(b_qkv, f)
    w_o = np.asarray(w_o, f)
    w1 = np.asarray(w1, f)
    w2 = np.asarray(w2, f)

    bc = lambda v, n=D: np.ascontiguousarray(
        np.broadcast_to(np.asarray(v, f).reshape(1, n), (P, n)))
    # w1/w2 in streaming-chunk SBUF layout
    w1h = np.ascontiguousarray(
        w1.reshape(DC, P, FC, P).transpose(2, 1, 0, 3)     # [fc, k, dc, m]
        .reshape(FC // 4, 4, P, DC, P).transpose(0, 2, 1, 3, 4)
        .astype(BFNP))
    w2h = np.ascontiguousarray(
        w2.reshape(FC, P, D).reshape(FC // 4, 4, P, D).transpose(0, 2, 1, 3)
        .astype(BFNP))
    shared = {
        "w1": w1h, "w2": w2h,
        "b1": np.ascontiguousarray(np.asarray(b1, f).reshape(FC, P).T),
        "b2b": bc(b2),
        "g1b": bc(g1), "be1b": bc(be1), "g2b": bc(g2), "be2b": bc(be2),
        "vones": np.ones((P, KT), f),
    }
    in_maps = []
    for c in range(8):
        n, r = divmod(c, 4)
        cols = slice(r * 256, (r + 1) * 256)
        xTn = np.ascontiguousarray(
            x[n].T.reshape(DC, P, L).transpose(1, 0, 2).astype(BFNP))
        rows = np.concatenate(
            [np.arange(q * QL + r * P, q * QL + (r + 1) * P)
             for q in range(QD)])
        xqn = np.ascontiguousarray(
            (x[n][rows] + np.asarray(b_o, f).reshape(1, D))
            .reshape(QD, P, D).transpose(1, 0, 2).astype(BFNP))
        m = dict(shared)
        m["xT"] = xTn
        m["xq"] = xqn
        m["wq"] = np.ascontiguousarray(
            w_qkv[:, :D][:, cols].reshape(DC, P, NP, P)
            .transpose(1, 2, 0, 3).astype(BFNP))
        m["wk"] = np.ascontiguousarray(
            w_qkv[:, D:2 * D][:, cols].reshape(DC, P, NP, P)
            .transpose(1, 2, 0, 3).astype(BFNP))
        m["wv"] = np.ascontiguousarray(
            w_qkv[:, 2 * D:][:, cols].reshape(DC, P, 2 * P)
            .transpose(1, 0, 2).astype(BFNP))
        m["wo"] = np.ascontiguousarray(
            w_o[cols, :].reshape(NP, P, D).transpose(1, 0, 2).astype(BFNP))
        m["bq"] = np.ascontiguousarray(b_qkv[:D][cols].reshape(NP, P).T)
        m["bk"] = np.ascontiguousarray(b_qkv[D:2 * D][cols].reshape(NP, P).T)
        m["bv"] = bc(b_qkv[2 * D:][cols], 2 * P)
        in_maps.append(m)
    return in_maps


def get_nc(affine=True):
    if affine not in _CACHED_NC:
        _CACHED_NC[affine] = _build_nc(affine)
    return _CACHED_NC[affine]


def kernel(**inputs):
    in_maps = make_in_maps(**inputs)
    affine = not (np.all(np.asarray(inputs["g1"]) == 1)
                  and np.all(np.asarray(inputs["be1"]) == 0)
                  and np.all(np.asarray(inputs["g2"]) == 1)
                  and np.all(np.asarray(inputs["be2"]) == 0))
    nc = get_nc(affine)
    # The axon-proxied NRT occasionally reports a transient
    # NRT_EXEC_UNIT_UNRECOVERABLE on a cold first dispatch; a plain retry
    # has always succeeded with bit-identical results, so recover inline.
    last_err = None
    for _ in range(3):
        try:
            res = run_bass_kernel_spmd(nc, in_maps, list(range(8))).results
            break
        except Exception as e:  # noqa: BLE001
            last_err = e
    else:
        raise last_err
    out = np.empty((NB, L, D), np.float32)
    for c in range(8):
        n, r = divmod(c, 4)
        yv = res[c]["y"]
        for q in range(QD):
            out[n, q * QL + r * P:q * QL + (r + 1) * P] = yv[q]
    return out


if __name__ == "__main__":
    rng = np.random.default_rng(0)
    demo = {
        "x": rng.standard_normal((NB, L, D)).astype(np.float32),
        "w_qkv": rng.standard_normal((D, 3 * D)).astype(np.float32) * 0.03,
        "b_qkv": rng.standard_normal(3 * D).astype(np.float32) * 0.03,
        "w_o": rng.standard_normal((D, D)).astype(np.float32) * 0.03,
        "b_o": rng.standard_normal(D).astype(np.float32) * 0.03,
        "g1": np.ones(D, np.float32), "be1": np.zeros(D, np.float32),
        "w1": rng.standard_normal((D, FF)).astype(np.float32) * 0.03,
        "b1": rng.standard_normal(FF).astype(np.float32) * 0.03,
        "w2": rng.standard_normal((FF, D)).astype(np.float32) * 0.015,
        "b2": rng.standard_normal(D).astype(np.float32) * 0.015,
        "g2": np.ones(D, np.float32), "be2": np.zeros(D, np.float32),
    }
    out = kernel(**demo)
    print("kernel output:", out.shape, out.dtype, np.abs(out).mean())


# revision 34
# speedup vs baseline: 1.1367x; 1.0870x over previous
"""Trainium2 Bass kernel for a transformer encoder layer (nn_Encoder).

x:[2,2048,1024] f32, 8 NeuronCores. Hybrid data/tensor parallel: core c
handles batch n=c//4 and head-group r=c%4 (4 of 16 heads). Each core
computes Q/K/V for its 4 heads over all 2048 tokens (no K/V recompute
redundancy), runs attention per 512-query quarter, then a partial output
projection; a per-quarter ReduceScatter over the 4-core group sums the
partials and hands each core 128 tokens per quarter (512 total) for the
LN1+FFN+LN2 tail. The 4 ReduceScatters run on the collective cores,
overlapped with attention of later quarters.

Matmul operands are bf16 (same PE rate as f32r, half the DMA/SBUF);
accumulation and the collective stay f32. LN stats in f32.
"""
import os
import sys

for _p in ("/opt/trn_rl_repo", "/root/.axon_site/_ro/trn_rl_repo"):
    if os.path.isdir(_p) and _p not in sys.path:
        sys.path.insert(0, _p)

import numpy as np
import ml_dtypes
import concourse.bass as bass
import concourse.mybir as mybir
import concourse.tile as tile
from concourse import bacc
from concourse.bass_utils import run_bass_kernel_spmd
from concourse.masks import make_identity

F32 = mybir.dt.float32
BF16 = mybir.dt.bfloat16
F8 = mybir.dt.float8e4
AF = mybir.ActivationFunctionType
ALU = mybir.AluOpType
BFNP = ml_dtypes.bfloat16
F8NP = mybir.dt.np(F8)
WS = 64.0           # host scale on w1/w2 so fp8e4m3 stays in normal range
WS2 = WS * WS       # folded into hb; LN2 is scale-invariant

D = 1024
H = 16
HD = 64
FF = 4096
L = 2048
NB = 2
P = 128
DC = D // P       # 8 chunks of the model dim
KT = L // P       # 16 key tiles
FC = FF // P      # 32 ff chunks
NP = 2            # head pairs per core (4 heads)
QD = 4            # query quarters
QL = L // QD      # 512 queries per quarter
EPS = 1e-5
GROUPS = [[0, 1, 2, 3], [4, 5, 6, 7]]

_CACHED_NC = {}


def _layernorm(nc, pool, dst, src, g_t, be_t, eps_t, zero_t, affine):
    """dst = (src - mean)/sqrt(var + eps) [* g + be], row-wise over 1024.

    var = E[x^2] - mu^2 (safe here: |mu| << rms). One Newton step refines
    the reciprocal sqrt.
    """
    mu = pool.tile([P, 1], F32, tag="ln_mu")
    nc.vector.tensor_reduce(mu[:], src, mybir.AxisListType.X, ALU.add)
    nc.vector.tensor_scalar_mul(mu[:], mu[:], 1.0 / D)
    c = pool.tile([P, D], F32, tag="ln_c")
    ss = pool.tile([P, 1], F32, tag="ln_ss")
    nc.scalar.activation(c[:], src, AF.Square, accum_out=ss[:])
    vv = pool.tile([P, 1], F32, tag="ln_v")
    nc.vector.tensor_scalar(vv[:], ss[:], 1.0 / D, EPS, ALU.mult, ALU.add)
    m2 = pool.tile([P, 1], F32, tag="ln_m2")
    nc.vector.tensor_tensor(m2[:], mu[:], mu[:], ALU.mult)
    nc.vector.tensor_tensor(vv[:], vv[:], m2[:], ALU.subtract)
    s = pool.tile([P, 1], F32, tag="ln_s")
    nc.scalar.activation(s[:], vv[:], AF.Sqrt, bias=zero_t[:])
    r = pool.tile([P, 1], F32, tag="ln_r")
    nc.vector.reciprocal(r[:], s[:])
    t = pool.tile([P, 1], F32, tag="ln_t")
    nc.vector.tensor_tensor(t[:], r[:], r[:], ALU.mult)
    nc.vector.tensor_tensor(t[:], t[:], vv[:], ALU.mult)
    nc.vector.tensor_scalar(t[:], t[:], -0.5, 1.5, ALU.mult, ALU.add)
    nc.vector.tensor_tensor(r[:], r[:], t[:], ALU.mult)
    nc.vector.tensor_scalar(dst, src, mu[:], r[:], ALU.subtract, ALU.mult)
    if affine:
        nc.vector.tensor_tensor(dst, dst, g_t[:], ALU.mult)
        nc.vector.tensor_tensor(dst, dst, be_t[:], ALU.add)


def _build_nc(affine=True):
    nc = bacc.Bacc("TRN2", target_bir_lowering=False, num_devices=8)

    def dparam(name, shape, dt=BF16):
        return nc.dram_tensor(name, shape, dt, kind="ExternalInput")

    xT = dparam("xT", [P, DC, L])           # x[n].T as [p, dc, t] (d=dc*128+p)
    xq = dparam("xq", [P, QD, D])           # owned token tiles, + b_o folded
    wq = dparam("wq", [P, NP, DC, P])       # [dpart, pair, dchunk, qcols]
    wk = dparam("wk", [P, NP, DC, P])
    wv = dparam("wv", [P, DC, 2 * P])       # [dpart, dchunk, vcols(4 heads)]
    wo = dparam("wo", [P, NP, D])           # [hd-part, pair, ocols]
    w1 = dparam("w1", [FC // 4, P, 4, 4, 2, P], F8)  # fc4-chunk, dcp-paired
    w2 = dparam("w2", [FC // 2, P, 2, D], F8)        # fcp chunk, slab-paired
    bq = dparam("bq", [P, NP], F32)
    bk = dparam("bk", [P, NP], F32)
    bv = dparam("bv", [P, 2 * P], F32)      # per-column bias, broadcast rows
    b1 = dparam("b1", [P, FC], F32)
    b2b = dparam("b2b", [P, D], F32)
    g1b = dparam("g1b", [P, D], F32)
    be1b = dparam("be1b", [P, D], F32)
    g2b = dparam("g2b", [P, D], F32)
    be2b = dparam("be2b", [P, D], F32)
    vones = dparam("vones", [P, KT], F32)

    parts = [nc.dram_tensor(f"part{q}", [QL, D], F32) for q in range(QD)]
    reds = [nc.dram_tensor(f"red{q}", [P, D], F32) for q in range(QD)]
    y = nc.dram_tensor("y", [QD, P, D], F32, kind="ExternalOutput")

    with tile.TileContext(nc) as tc:
        with tc.tile_pool(name="pers", bufs=1) as pers, \
             tc.tile_pool(name="wp", bufs=1) as wp, \
             tc.tile_pool(name="stg", bufs=2) as stg, \
             tc.tile_pool(name="atp", bufs=3) as atp, \
             tc.tile_pool(name="atd", bufs=2) as atd, \
             tc.tile_pool(name="hpp", bufs=2) as hpp, \
             tc.tile_pool(name="lnp", bufs=2) as lnp, \
             tc.tile_pool(name="fp", bufs=2) as fp, \
             tc.tile_pool(name="fp2", bufs=2) as fp2, \
             tc.tile_pool(name="t2p", bufs=2) as t2p, \
             tc.tile_pool(name="ps", bufs=1, space="PSUM") as ps:

            # ---- persistent SBUF ----
            xT_t = pers.tile([P, DC, L], BF16, tag="blobA")      # 32KB
            qT_t = pers.tile([P, NP, L], BF16, tag="qT")
            kT_t = pers.tile([P, NP, L], BF16, tag="kT")
            v_aug = pers.tile([P, KT, 4, HD + 1], BF16, tag="vaug")
            outSB = pers.tile([P, NP, QL], BF16, tag="outSB")
            h_t = pers.tile([P, QD, D], F32, tag="h")            # post-LN1
            hb_t = pers.tile([P, QD, D], F32, tag="hb")          # h + b2
            hT_t = pers.tile([P, DC, QD * P], F8, tag="hT")
            xq_t = pers.tile([P, QD, D], BF16, tag="xq")
            eps_t = pers.tile([P, 1], F32, tag="eps")
            zero_t = pers.tile([P, 1], F32, tag="zero")
            ident = pers.tile([P, P], F32, tag="ident")
            nc.gpsimd.memset(eps_t[:], EPS)
            nc.gpsimd.memset(zero_t[:], 0.0)
            make_identity(nc, ident[:])

            # psum tiles: fixed 8-bank plan, manually assigned per phase
            def pt(tag, name, cols=1024):
                return ps.tile([P, cols], F32, tag=tag, name=name)

            # ---- weight / bias prefetch (scalar queue) ----
            wq_t = wp.tile([P, NP, DC, P], BF16, tag="wq")
            wk_t = wp.tile([P, NP, DC, P], BF16, tag="wk")
            wv_t = wp.tile([P, DC, 2 * P], BF16, tag="wv")
            wo_t = wp.tile([P, NP, D], BF16, tag="wo")
            bq_t = wp.tile([P, NP], F32, tag="bq")
            bk_t = wp.tile([P, NP], F32, tag="bk")
            bv_t = wp.tile([P, 2 * P], F32, tag="bv")
            b1_t = wp.tile([P, FC], F32, tag="b1")
            b2b_t = wp.tile([P, D], F32, tag="b2b")
            ones_t = wp.tile([P, KT], F32, tag="ones")
            # wq pair0 first so the very first Q matmul can start sooner
            nc.scalar.dma_start(wq_t[:, 0], wq[:, 0])
            nc.scalar.dma_start(wq_t[:, 1], wq[:, 1])
            nc.scalar.dma_start(wk_t[:], wk[:])
            nc.scalar.dma_start(wv_t[:], wv[:])
            nc.scalar.dma_start(wo_t[:], wo[:])
            nc.scalar.dma_start(bq_t[:], bq[:])
            nc.scalar.dma_start(bk_t[:], bk[:])
            nc.scalar.dma_start(bv_t[:], bv[:])
            nc.scalar.dma_start(b1_t[:], b1[:])
            nc.scalar.dma_start(b2b_t[:], b2b[:])
            nc.scalar.dma_start(ones_t[:], vones[:])
            if affine:
                g1b_t = wp.tile([P, D], F32, tag="g1b")
                be1b_t = wp.tile([P, D], F32, tag="be1b")
                g2b_t = wp.tile([P, D], F32, tag="g2b")
                be2b_t = wp.tile([P, D], F32, tag="be2b")
                nc.scalar.dma_start(g1b_t[:], g1b[:])
                nc.scalar.dma_start(be1b_t[:], be1b[:])
                nc.scalar.dma_start(g2b_t[:], g2b[:])
                nc.scalar.dma_start(be2b_t[:], be2b[:])
            else:
                g1b_t = be1b_t = g2b_t = be2b_t = None

            # x stream (sync queue), per-dc so Q proj starts on chunk 0
            for dc in range(DC):
                nc.sync.dma_start(xT_t[:, dc, :], xT[:, dc, :])
            nc.sync.dma_start(xq_t[:], xq[:])

            # 8 half-bank accumulation slots for the projection phases.
            # One tile per tag per phase; slots address halves explicitly
            # (repeated same-tag tile creation would WAW-serialize).
            def phase_slots(phase):
                tiles = {t: pt(t, f"{phase}_{t}",
                               cols=512 if t in ("pE", "pF") else 1024)
                         for t in ("pAB", "pCD", "pGH", "pE", "pF")}
                layout = [("pAB", 0), ("pAB", 512), ("pCD", 0), ("pCD", 512),
                          ("pGH", 0), ("pGH", 512), ("pE", 0), ("pF", 0)]
                return [(tiles[t], off) for t, off in layout]

            # ================= Q projection (dc-outer sweeps) ==========
            # dc-outer so the first matmuls ride the xT chunk stream.
            qsl = phase_slots("q")
            for pair in range(NP):
                for dc in range(DC):
                    for qt in range(4):
                        tile_, off = qsl[pair * 4 + qt]
                        nc.tensor.matmul(tile_[:, off:off + 512],
                                         wq_t[:, pair, dc, :],
                                         xT_t[:, dc, qt * 512:(qt + 1) * 512],
                                         start=(dc == 0), stop=(dc == DC - 1))
                for qt in range(4):
                    tile_, off = qsl[pair * 4 + qt]
                    nc.vector.tensor_scalar(
                        qT_t[:, pair, qt * 512:(qt + 1) * 512],
                        tile_[:, off:off + 512],
                        bq_t[:, pair:pair + 1], None, ALU.add)

            # ================= K projection (dc-inner) =================
            ksl = phase_slots("k")
            for pair in range(NP):
                for qt in range(4):
                    i = pair * 4 + qt
                    tile_, off = ksl[i]
                    for dc in range(DC):
                        nc.tensor.matmul(tile_[:, off:off + 512],
                                         wk_t[:, pair, dc, :],
                                         xT_t[:, dc, qt * 512:(qt + 1) * 512],
                                         start=(dc == 0), stop=(dc == DC - 1))
                    nc.vector.tensor_scalar(
                        kT_t[:, pair, qt * 512:(qt + 1) * 512],
                        tile_[:, off:off + 512],
                        bk_t[:, pair:pair + 1], None, ALU.add)

            # ================= V projection ============================
            nc.vector.tensor_copy(
                v_aug[:, :, :, HD],
                ones_t[:, :, None].to_broadcast([P, KT, 4]))
            vsl = phase_slots("v")
            for tt in range(KT):
                tile_, off = vsl[tt % 8]
                for dc in range(DC):
                    nc.tensor.matmul(tile_[:, off:off + 256],
                                     xT_t[:, dc, tt * P:(tt + 1) * P],
                                     wv_t[:, dc, :],
                                     start=(dc == 0), stop=(dc == DC - 1))
                nc.vector.tensor_tensor(
                    v_aug[:, tt, :, 0:HD],
                    tile_[:, off:off + 256].rearrange("p (h c) -> p h c", c=HD),
                    bv_t[:].rearrange("p (h c) -> p h c", c=HD),
                    ALU.add)

            # ====== attention + out-proj partial + RS, per quarter =====
            def post_quarter(qd):
                """RS result -> +x residual -> LN1 (transpose issued apart)."""
                hpre = hpp.tile([P, D], F32, tag="hpre", name=f"hpre_{qd}")
                nc.sync.dma_start(hpre[:], reds[qd][:])
                nc.vector.tensor_tensor(hpre[:], hpre[:], xq_t[:, qd, :],
                                        ALU.add)
                _layernorm(nc, lnp, h_t[:, qd, :], hpre[:], g1b_t, be1b_t,
                           eps_t, zero_t, affine)
                nc.vector.tensor_scalar_mul(hb_t[:, qd, :], h_t[:, qd, :],
                                            WS2)
                nc.vector.tensor_tensor(hb_t[:, qd, :], hb_t[:, qd, :],
                                        b2b_t[:], ALU.add)

            def pe_transpose(qd):
                """h_t[:, qd, :] -> hT_t[:, :, qd*P:...] via 8 PE transposes
                through a pGH psum tile, drained by the Pool engine."""
                trp = pt("pGH", f"tr_{qd}")
                for dc in range(DC):
                    nc.tensor.transpose(trp[:, dc * P:(dc + 1) * P],
                                        h_t[:, qd, dc * P:(dc + 1) * P],
                                        ident[:])
                nc.vector.tensor_copy(
                    hT_t[:, :, qd * P:(qd + 1) * P],
                    trp[:].rearrange("p (c t) -> p c t", t=P))

            for qd in range(QD):
                qcols = slice(qd * QL, (qd + 1) * QL)
                # ---- attention for 2 pairs ----
                stprot = ["pAB", "pCD"]
                sti = 0
                for pair in range(NP):
                    pvs = [pt("pE", f"pv_{qd}_{pair}_0", cols=512),
                           pt("pF", f"pv_{qd}_{pair}_1", cols=512)]
                    for ktp in range(KT // 2):
                        # S for both heads first, then both PVs; the stp psum
                        # rotates over 3 double-bank slots so the PE can run
                        # several S groups ahead of the ACT exp stream.
                        ptts = []
                        for h2 in range(2):
                            rows = slice(h2 * HD, h2 * HD + HD)
                            stp = pt(stprot[sti % 2],
                                     f"st_{qd}_{pair}_{ktp}_{h2}")
                            sti += 1
                            for j in range(2):
                                kt = 2 * ktp + j
                                nc.tensor.matmul(
                                    stp[:, j * 512:(j + 1) * 512],
                                    kT_t[rows, pair, kt * P:(kt + 1) * P],
                                    qT_t[rows, pair, qcols],
                                    start=True, stop=True)
                            ptt = atp.tile([P, 2, 512], BF16, tag="pt",
                                           name=f"pt_{qd}_{pair}_{ktp}_{h2}")
                            nc.scalar.activation(
                                ptt[:],
                                stp[:].rearrange("p (c n) -> p c n", n=512),
                                AF.Exp, scale=0.125)
                            ptts.append(ptt)
                        for h2 in range(2):
                            for j in range(2):
                                kt = 2 * ktp + j
                                nc.tensor.matmul(
                                    pvs[h2][:HD + 1, :],
                                    v_aug[:, kt, 2 * pair + h2, :],
                                    ptts[h2][:, j, :],
                                    start=(kt == 0), stop=(kt == KT - 1))
                    for h2 in range(2):
                        rows = slice(h2 * HD, h2 * HD + HD)
                        den = atd.tile([1, QL], F32, tag="den",
                                       name=f"den_{qd}_{pair}_{h2}")
                        nc.vector.reciprocal(den[:], pvs[h2][HD:HD + 1, :])
                        denb = atd.tile([HD, QL], F32, tag="denb",
                                        name=f"denb_{qd}_{pair}_{h2}")
                        nc.gpsimd.partition_broadcast(denb[:], den[:])
                        nc.vector.tensor_tensor(outSB[rows, pair, :],
                                                pvs[h2][:HD, :], denb[:],
                                                ALU.mult)

                # ---- out-proj partial: [512 q, 1024], psum->SBUF->DRAM ----
                poslots = ["pGH", "pAB", "pCD", "pGH"]
                for tt4 in range(4):
                    po = pt(poslots[tt4], f"po_{qd}_{tt4}")
                    for pair in range(NP):
                        for oc in range(2):
                            nc.tensor.matmul(
                                po[:, oc * 512:(oc + 1) * 512],
                                outSB[:, pair, tt4 * P:(tt4 + 1) * P],
                                wo_t[:, pair, oc * 512:(oc + 1) * 512],
                                start=(pair == 0), stop=(pair == NP - 1))
                    pst = stg.tile([P, D], F32, tag="pstg",
                                   name=f"pstg_{qd}_{tt4}")
                    # drain on both DVE and ACT: ACT has a lull at quarter
                    # end while the next quarter's S matmuls prime the exps
                    if tt4 % 2 == 0:
                        nc.vector.tensor_copy(pst[:], po[:])
                    else:
                        nc.scalar.activation(pst[:], po[:], AF.Copy)
                    nc.sync.dma_start(
                        parts[qd].rearrange("(a p) d -> p a d", p=P)[:, tt4, :],
                        pst[:])
                nc.gpsimd.collective_compute(
                    "ReduceScatter", ALU.add, replica_groups=GROUPS,
                    ins=[parts[qd][:].opt()], outs=[reds[qd][:].opt()])
            # posts AFTER the full attention loop, with scheduler hints so
            # the greedy per-engine scheduler does not slot their long
            # collective-gated waits into the attention instruction streams
            # (a wait at a queue head blocks everything behind it).
            for qd in range(QD):
                with tc.tile_wait_until((0.138, 0.185, 0.235, 0.285)[qd]):
                    post_quarter(qd)
            for qd in range(QD - 1):
                pe_transpose(qd)

            # ================= FFN =====================================
            # Four phases: fc1(c1) -> fc2(c1) -> fc1(c2) -> fc2(c2).
            # fc2(c1) only needs ff1 for chunk-1 tokens, so it covers the
            # RS_q3 -> LN1_q3 -> transpose latency that gates fc1(c2).
            # w1 and w2 each stream twice (bf16, ~270 GB/s bursts).
            ff1 = pers.tile([P, FC, QD * P], F8, tag="blobA")

            def fc1_chunk(ch):
                tcols = slice(ch * 256, (ch + 1) * 256)
                f1AB = pt("pAB", f"f1AB_{ch}")
                f1CD = pt("pCD", f"f1CD_{ch}")
                f1slots = [(f1AB, 0), (f1AB, 512), (f1CD, 0), (f1CD, 512)]
                for fc4 in range(FC // 4):
                    w1_t = fp.tile([P, 4, 4, 2, P], F8, tag="w1s")
                    nc.sync.dma_start(w1_t[:], w1[fc4])
                    for f in range(4):
                        fc = 4 * fc4 + f
                        tile_, sl = f1slots[f]
                        for dcp in range(4):
                            nc.tensor.matmul(
                                tile_[:, sl:sl + 256],
                                w1_t[:, f, dcp],
                                hT_t[:, 2 * dcp:2 * dcp + 2, tcols],
                                start=(dcp == 0), stop=(dcp == 3),
                                perf_mode=mybir.MatmulPerfMode.DoubleRow)
                        nc.vector.tensor_scalar(
                            ff1[:, fc, tcols], tile_[:, sl:sl + 256],
                            b1_t[:, fc:fc + 1], 0.0, ALU.add, ALU.max)

            def fc2_chunk(ch):
                tts = (2 * ch, 2 * ch + 1)
                tGH = pt("pGH", f"f2GH_{ch}")
                tE = pt("pE", f"f2E_{ch}", cols=512)
                tF = pt("pF", f"f2F_{ch}", cols=512)
                dsts = {(tts[0], 0): tGH[:, 0:512], (tts[0], 1): tGH[:, 512:1024],
                        (tts[1], 0): tE[:, :], (tts[1], 1): tF[:, :]}
                for fcp in range(FC // 2):
                    w2_t = fp2.tile([P, 2, D], F8, tag="w2s")
                    nc.sync.dma_start(w2_t[:], w2[fcp])
                    st = (fcp == 0)
                    sp = (fcp == FC // 2 - 1)
                    for tt in tts:
                        for oc in range(2):
                            nc.tensor.matmul(
                                dsts[(tt, oc)],
                                ff1[:, 2 * fcp:2 * fcp + 2,
                                    tt * P:(tt + 1) * P],
                                w2_t[:, :, oc * 512:(oc + 1) * 512],
                                start=st, stop=sp,
                                perf_mode=mybir.MatmulPerfMode.DoubleRow)
                for i, tt in enumerate(tts):
                    t2 = t2p.tile([P, D], F32, tag="t2", name=f"t2_{tt}")
                    if i == 0:
                        nc.vector.tensor_tensor(t2[:], tGH[:],
                                                hb_t[:, tt, :], ALU.add)
                    else:
                        nc.vector.tensor_tensor(t2[:, 0:512], tE[:],
                                                hb_t[:, tt, 0:512], ALU.add)
                        nc.vector.tensor_tensor(t2[:, 512:1024], tF[:],
                                                hb_t[:, tt, 512:1024], ALU.add)
                    _layernorm(nc, lnp, t2[:], t2[:], g2b_t, be2b_t, eps_t,
                               zero_t, affine)
                    nc.sync.dma_start(y[tt], t2[:])

            fc1_chunk(0)
            fc2_chunk(0)
            pe_transpose(QD - 1)
            fc1_chunk(1)
            fc2_chunk(1)

    nc.compile()
    return nc


def make_in_maps(x, w_qkv, b_qkv, w_o, b_o, g1, be1, w1, b1, w2, b2, g2, be2):
    f = np.float32
    x = np.asarray(x, f)
    w_qkv = np.asarray(w_qkv, f)
    b_qkv = np.asarray(b_qkv, f)
    w_o = np.asarray(w_o, f)
    w1 = np.asarray(w1, f)
    w2 = np.asarray(w2, f)

    bc = lambda v, n=D: np.ascontiguousarray(
        np.broadcast_to(np.asarray(v, f).reshape(1, n), (P, n)))
    # w1/w2 in streaming-chunk SBUF layout
    w1h = np.ascontiguousarray(
        (w1 * WS).reshape(DC, P, FC, P).transpose(2, 1, 0, 3)  # [fc, k, dc, m]
        .reshape(FC // 4, 4, P, 4, 2, P).transpose(0, 2, 1, 3, 4, 5)
        .astype(F8NP))
    w2h = np.ascontiguousarray(
        (w2 * WS).reshape(FC, P, D).reshape(FC // 2, 2, P, D)
        .transpose(0, 2, 1, 3).astype(F8NP))
    shared = {
        "w1": w1h, "w2": w2h,
        "b1": np.ascontiguousarray(np.asarray(b1, f).reshape(FC, P).T * WS),
        "b2b": bc(np.asarray(b2, f) * WS2),
        "g1b": bc(g1), "be1b": bc(be1), "g2b": bc(g2), "be2b": bc(be2),
        "vones": np.ones((P, KT), f),
    }
    in_maps = []
    for c in range(8):
        n, r = divmod(c, 4)
        cols = slice(r * 256, (r + 1) * 256)
        xTn = np.ascontiguousarray(
            x[n].T.reshape(DC, P, L).transpose(1, 0, 2).astype(BFNP))
        rows = np.concatenate(
            [np.arange(q * QL + r * P, q * QL + (r + 1) * P)
             for q in range(QD)])
        xqn = np.ascontiguousarray(
            (x[n][rows] + np.asarray(b_o, f).reshape(1, D))
            .reshape(QD, P, D).transpose(1, 0, 2).astype(BFNP))
        m = dict(shared)
        m["xT"] = xTn
        m["xq"] = xqn
        m["wq"] = np.ascontiguousarray(
            w_qkv[:, :D][:, cols].reshape(DC, P, NP, P)
            .transpose(1, 2, 0, 3).astype(BFNP))
        m["wk"] = np.ascontiguousarray(
            w_qkv[:, D:2 * D][:, cols].reshape(DC, P, NP, P)
            .transpose(1, 2, 0, 3).astype(BFNP))
        m["wv"] = np.ascontiguousarray(
            w_qkv[:, 2 * D:][:, cols].reshape(DC, P, 2 * P)
            .transpose(1, 0, 2).astype(BFNP))
        m["wo"] = np.ascontiguousarray(
            w_o[cols, :].reshape(NP, P, D).transpose(1, 0, 2).astype(BFNP))
        m["bq"] = np.ascontiguousarray(b_qkv[:D][cols].reshape(NP, P).T)
        m["bk"] = np.ascontiguousarray(b_qkv[D:2 * D][cols].reshape(NP, P).T)
        m["bv"] = bc(b_qkv[2 * D:][cols], 2 * P)
        in_maps.append(m)
    return in_maps


def get_nc(affine=True):
    if affine not in _CACHED_NC:
        _CACHED_NC[affine] = _build_nc(affine)
    return _CACHED_NC[affine]


def kernel(**inputs):
    in_maps = make_in_maps(**inputs)
    affine = not (np.all(np.asarray(inputs["g1"]) == 1)
                  and np.all(np.asarray(inputs["be1"]) == 0)
                  and np.all(np.asarray(inputs["g2"]) == 1)
                  and np.all(np.asarray(inputs["be2"]) == 0))
    nc = get_nc(affine)
    # The axon-proxied NRT occasionally reports a transient
    # NRT_EXEC_UNIT_UNRECOVERABLE on a cold first dispatch; a plain retry
    # has always succeeded with bit-identical results, so recover inline.
    last_err = None
    for _ in range(3):
        try:
            res = run_bass_kernel_spmd(nc, in_maps, list(range(8))).results
            break
        except Exception as e:  # noqa: BLE001
            last_err = e
    else:
        raise last_err
    out = np.empty((NB, L, D), np.float32)
    for c in range(8):
        n, r = divmod(c, 4)
        yv = res[c]["y"]
        for q in range(QD):
            out[n, q * QL + r * P:q * QL + (r + 1) * P] = yv[q]
    return out


if __name__ == "__main__":
    rng = np.random.default_rng(0)
    demo = {
        "x": rng.standard_normal((NB, L, D)).astype(np.float32),
        "w_qkv": rng.standard_normal((D, 3 * D)).astype(np.float32) * 0.03,
        "b_qkv": rng.standard_normal(3 * D).astype(np.float32) * 0.03,
        "w_o": rng.standard_normal((D, D)).astype(np.float32) * 0.03,
        "b_o": rng.standard_normal(D).astype(np.float32) * 0.03,
        "g1": np.ones(D, np.float32), "be1": np.zeros(D, np.float32),
        "w1": rng.standard_normal((D, FF)).astype(np.float32) * 0.03,
        "b1": rng.standard_normal(FF).astype(np.float32) * 0.03,
        "w2": rng.standard_normal((FF, D)).astype(np.float32) * 0.015,
        "b2": rng.standard_normal(D).astype(np.float32) * 0.015,
        "g2": np.ones(D, np.float32), "be2": np.zeros(D, np.float32),
    }
    out = kernel(**demo)
    print("kernel output:", out.shape, out.dtype, np.abs(out).mean())


# revision 35
# speedup vs baseline: 1.1521x; 1.0135x over previous
"""Trainium2 Bass kernel for a transformer encoder layer (nn_Encoder).

x:[2,2048,1024] f32, 8 NeuronCores. Hybrid data/tensor parallel: core c
handles batch n=c//4 and head-group r=c%4 (4 of 16 heads). Each core
computes Q/K/V for its 4 heads over all 2048 tokens (no K/V recompute
redundancy), runs attention per 512-query quarter, then a partial output
projection; a per-quarter ReduceScatter over the 4-core group sums the
partials and hands each core 128 tokens per quarter (512 total) for the
LN1+FFN+LN2 tail. The 4 ReduceScatters run on the collective cores,
overlapped with attention of later quarters.

Matmul operands are bf16 (same PE rate as f32r, half the DMA/SBUF);
accumulation and the collective stay f32. LN stats in f32.
"""
import os
import sys

for _p in ("/opt/trn_rl_repo", "/root/.axon_site/_ro/trn_rl_repo"):
    if os.path.isdir(_p) and _p not in sys.path:
        sys.path.insert(0, _p)

import numpy as np
import ml_dtypes
import concourse.bass as bass
import concourse.mybir as mybir
import concourse.tile as tile
from concourse import bacc
from concourse.bass_utils import run_bass_kernel_spmd
from concourse.masks import make_identity

F32 = mybir.dt.float32
BF16 = mybir.dt.bfloat16
F8 = mybir.dt.float8e4
AF = mybir.ActivationFunctionType
ALU = mybir.AluOpType
BFNP = ml_dtypes.bfloat16
F8NP = mybir.dt.np(F8)
WS = 64.0           # host scale on w1/w2 so fp8e4m3 stays in normal range
WS2 = WS * WS       # folded into hb; LN2 is scale-invariant

D = 1024
H = 16
HD = 64
FF = 4096
L = 2048
NB = 2
P = 128
DC = D // P       # 8 chunks of the model dim
KT = L // P       # 16 key tiles
FC = FF // P      # 32 ff chunks
NP = 2            # head pairs per core (4 heads)
QD = 4            # query quarters
QL = L // QD      # 512 queries per quarter
EPS = 1e-5
GROUPS = [[0, 1, 2, 3], [4, 5, 6, 7]]

_CACHED_NC = {}


def _layernorm(nc, pool, dst, src, g_t, be_t, eps_t, zero_t, affine):
    """dst = (src - mean)/sqrt(var + eps) [* g + be], row-wise over 1024.

    var = E[x^2] - mu^2 (safe here: |mu| << rms). One Newton step refines
    the reciprocal sqrt.
    """
    mu = pool.tile([P, 1], F32, tag="ln_mu")
    nc.vector.tensor_reduce(mu[:], src, mybir.AxisListType.X, ALU.add)
    nc.vector.tensor_scalar_mul(mu[:], mu[:], 1.0 / D)
    c = pool.tile([P, D], F32, tag="ln_c")
    ss = pool.tile([P, 1], F32, tag="ln_ss")
    nc.scalar.activation(c[:], src, AF.Square, accum_out=ss[:])
    vv = pool.tile([P, 1], F32, tag="ln_v")
    nc.vector.tensor_scalar(vv[:], ss[:], 1.0 / D, EPS, ALU.mult, ALU.add)
    m2 = pool.tile([P, 1], F32, tag="ln_m2")
    nc.vector.tensor_tensor(m2[:], mu[:], mu[:], ALU.mult)
    nc.vector.tensor_tensor(vv[:], vv[:], m2[:], ALU.subtract)
    s = pool.tile([P, 1], F32, tag="ln_s")
    nc.scalar.activation(s[:], vv[:], AF.Sqrt, bias=zero_t[:])
    r = pool.tile([P, 1], F32, tag="ln_r")
    nc.vector.reciprocal(r[:], s[:])
    t = pool.tile([P, 1], F32, tag="ln_t")
    nc.vector.tensor_tensor(t[:], r[:], r[:], ALU.mult)
    nc.vector.tensor_tensor(t[:], t[:], vv[:], ALU.mult)
    nc.vector.tensor_scalar(t[:], t[:], -0.5, 1.5, ALU.mult, ALU.add)
    nc.vector.tensor_tensor(r[:], r[:], t[:], ALU.mult)
    nc.vector.tensor_scalar(dst, src, mu[:], r[:], ALU.subtract, ALU.mult)
    if affine:
        nc.vector.tensor_tensor(dst, dst, g_t[:], ALU.mult)
        nc.vector.tensor_tensor(dst, dst, be_t[:], ALU.add)


def _build_nc(affine=True):
    nc = bacc.Bacc("TRN2", target_bir_lowering=False, num_devices=8)

    def dparam(name, shape, dt=BF16):
        return nc.dram_tensor(name, shape, dt, kind="ExternalInput")

    xT = dparam("xT", [P, DC, L], F8)       # x[n].T as [p, dc, t] (d=dc*128+p)
    xq = dparam("xq", [P, QD, D])           # owned token tiles, + b_o folded
    wq = dparam("wq", [P, NP, 4, 2, P], F8)  # [dpart, pair, dcp, slab, qcols]
    wk = dparam("wk", [P, NP, 4, 2, P], F8)
    wv = dparam("wv", [P, 4, 2, 2 * P], F8)  # [dpart, dcp, slab, vcols]
    wo = dparam("wo", [P, NP, D])           # [hd-part, pair, ocols]
    w1 = dparam("w1", [FC // 4, P, 4, 4, 2, P], F8)  # fc4-chunk, dcp-paired
    w2 = dparam("w2", [FC // 2, P, 2, D], F8)        # fcp chunk, slab-paired
    bq = dparam("bq", [P, NP], F32)
    bk = dparam("bk", [P, NP], F32)
    bv = dparam("bv", [P, 2 * P], F32)      # per-column bias, broadcast rows
    b1 = dparam("b1", [P, FC], F32)
    b2b = dparam("b2b", [P, D], F32)
    g1b = dparam("g1b", [P, D], F32)
    be1b = dparam("be1b", [P, D], F32)
    g2b = dparam("g2b", [P, D], F32)
    be2b = dparam("be2b", [P, D], F32)
    vones = dparam("vones", [P, KT], F32)

    parts = [nc.dram_tensor(f"part{q}", [QL, D], F32) for q in range(QD)]
    reds = [nc.dram_tensor(f"red{q}", [P, D], F32) for q in range(QD)]
    y = nc.dram_tensor("y", [QD, P, D], F32, kind="ExternalOutput")

    with tile.TileContext(nc) as tc:
        with tc.tile_pool(name="pers", bufs=1) as pers, \
             tc.tile_pool(name="wp", bufs=1) as wp, \
             tc.tile_pool(name="stg", bufs=2) as stg, \
             tc.tile_pool(name="atp", bufs=3) as atp, \
             tc.tile_pool(name="atd", bufs=2) as atd, \
             tc.tile_pool(name="hpp", bufs=2) as hpp, \
             tc.tile_pool(name="lnp", bufs=2) as lnp, \
             tc.tile_pool(name="fp", bufs=2) as fp, \
             tc.tile_pool(name="fp2", bufs=2) as fp2, \
             tc.tile_pool(name="t2p", bufs=2) as t2p, \
             tc.tile_pool(name="ps", bufs=1, space="PSUM") as ps:

            # ---- persistent SBUF ----
            xT_t = pers.tile([P, DC, L], F8, tag="blobA")        # 16KB
            qT_t = pers.tile([P, NP, L], BF16, tag="qT")
            kT_t = pers.tile([P, NP, L], BF16, tag="kT")
            v_aug = pers.tile([P, KT, 4, HD + 1], BF16, tag="vaug")
            outSB = pers.tile([P, NP, QL], BF16, tag="outSB")
            h_t = pers.tile([P, QD, D], F32, tag="h")            # post-LN1
            hb_t = pers.tile([P, QD, D], F32, tag="hb")          # h + b2
            hT_t = pers.tile([P, DC, QD * P], F8, tag="hT")
            xq_t = pers.tile([P, QD, D], BF16, tag="xq")
            eps_t = pers.tile([P, 1], F32, tag="eps")
            zero_t = pers.tile([P, 1], F32, tag="zero")
            ident = pers.tile([P, P], F32, tag="ident")
            nc.gpsimd.memset(eps_t[:], EPS)
            nc.gpsimd.memset(zero_t[:], 0.0)
            make_identity(nc, ident[:])

            # psum tiles: fixed 8-bank plan, manually assigned per phase
            def pt(tag, name, cols=1024):
                return ps.tile([P, cols], F32, tag=tag, name=name)

            # ---- weight / bias prefetch (scalar queue) ----
            wq_t = wp.tile([P, NP, 4, 2, P], F8, tag="wq")
            wk_t = wp.tile([P, NP, 4, 2, P], F8, tag="wk")
            wv_t = wp.tile([P, 4, 2, 2 * P], F8, tag="wv")
            wo_t = wp.tile([P, NP, D], BF16, tag="wo")
            bq_t = wp.tile([P, NP], F32, tag="bq")
            bk_t = wp.tile([P, NP], F32, tag="bk")
            bv_t = wp.tile([P, 2 * P], F32, tag="bv")
            b1_t = wp.tile([P, FC], F32, tag="b1")
            b2b_t = wp.tile([P, D], F32, tag="b2b")
            ones_t = wp.tile([P, KT], F32, tag="ones")
            # wq pair0 first so the very first Q matmul can start sooner
            nc.scalar.dma_start(wq_t[:, 0], wq[:, 0])
            nc.scalar.dma_start(wq_t[:, 1], wq[:, 1])
            nc.scalar.dma_start(wk_t[:], wk[:])
            nc.scalar.dma_start(wv_t[:], wv[:])
            nc.scalar.dma_start(wo_t[:], wo[:])
            nc.scalar.dma_start(bq_t[:], bq[:])
            nc.scalar.dma_start(bk_t[:], bk[:])
            nc.scalar.dma_start(bv_t[:], bv[:])
            nc.scalar.dma_start(b1_t[:], b1[:])
            nc.scalar.dma_start(b2b_t[:], b2b[:])
            nc.scalar.dma_start(ones_t[:], vones[:])
            if affine:
                g1b_t = wp.tile([P, D], F32, tag="g1b")
                be1b_t = wp.tile([P, D], F32, tag="be1b")
                g2b_t = wp.tile([P, D], F32, tag="g2b")
                be2b_t = wp.tile([P, D], F32, tag="be2b")
                nc.scalar.dma_start(g1b_t[:], g1b[:])
                nc.scalar.dma_start(be1b_t[:], be1b[:])
                nc.scalar.dma_start(g2b_t[:], g2b[:])
                nc.scalar.dma_start(be2b_t[:], be2b[:])
            else:
                g1b_t = be1b_t = g2b_t = be2b_t = None

            # x stream (sync queue), per-dc so Q proj starts on chunk 0
            for dc in range(DC):
                nc.sync.dma_start(xT_t[:, dc, :], xT[:, dc, :])
            nc.sync.dma_start(xq_t[:], xq[:])

            # 8 half-bank accumulation slots for the projection phases.
            # One tile per tag per phase; slots address halves explicitly
            # (repeated same-tag tile creation would WAW-serialize).
            def phase_slots(phase):
                tiles = {t: pt(t, f"{phase}_{t}",
                               cols=512 if t in ("pE", "pF") else 1024)
                         for t in ("pAB", "pCD", "pGH", "pE", "pF")}
                layout = [("pAB", 0), ("pAB", 512), ("pCD", 0), ("pCD", 512),
                          ("pGH", 0), ("pGH", 512), ("pE", 0), ("pF", 0)]
                return [(tiles[t], off) for t, off in layout]

            # ================= Q projection (dc-outer sweeps) ==========
            # dc-outer so the first matmuls ride the xT chunk stream.
            qsl = phase_slots("q")
            for pair in range(NP):
                for dcp in range(4):
                    for qt in range(4):
                        tile_, off = qsl[pair * 4 + qt]
                        nc.tensor.matmul(
                            tile_[:, off:off + 512],
                            wq_t[:, pair, dcp],
                            xT_t[:, 2 * dcp:2 * dcp + 2,
                                 qt * 512:(qt + 1) * 512],
                            start=(dcp == 0), stop=(dcp == 3),
                            perf_mode=mybir.MatmulPerfMode.DoubleRow)
                for qt in range(4):
                    tile_, off = qsl[pair * 4 + qt]
                    nc.vector.tensor_scalar(
                        qT_t[:, pair, qt * 512:(qt + 1) * 512],
                        tile_[:, off:off + 512],
                        bq_t[:, pair:pair + 1], None, ALU.add)

            # ================= K projection (dc-inner) =================
            ksl = phase_slots("k")
            for pair in range(NP):
                for qt in range(4):
                    i = pair * 4 + qt
                    tile_, off = ksl[i]
                    for dcp in range(4):
                        nc.tensor.matmul(
                            tile_[:, off:off + 512],
                            wk_t[:, pair, dcp],
                            xT_t[:, 2 * dcp:2 * dcp + 2,
                                 qt * 512:(qt + 1) * 512],
                            start=(dcp == 0), stop=(dcp == 3),
                            perf_mode=mybir.MatmulPerfMode.DoubleRow)
                    nc.vector.tensor_scalar(
                        kT_t[:, pair, qt * 512:(qt + 1) * 512],
                        tile_[:, off:off + 512],
                        bk_t[:, pair:pair + 1], None, ALU.add)

            # ================= V projection ============================
            nc.vector.tensor_copy(
                v_aug[:, :, :, HD],
                ones_t[:, :, None].to_broadcast([P, KT, 4]))
            vsl = phase_slots("v")
            for tt in range(KT):
                tile_, off = vsl[tt % 8]
                for dcp in range(4):
                    nc.tensor.matmul(
                        tile_[:, off:off + 256],
                        xT_t[:, 2 * dcp:2 * dcp + 2, tt * P:(tt + 1) * P],
                        wv_t[:, dcp],
                        start=(dcp == 0), stop=(dcp == 3),
                        perf_mode=mybir.MatmulPerfMode.DoubleRow)
                nc.vector.tensor_tensor(
                    v_aug[:, tt, :, 0:HD],
                    tile_[:, off:off + 256].rearrange("p (h c) -> p h c", c=HD),
                    bv_t[:].rearrange("p (h c) -> p h c", c=HD),
                    ALU.add)

            # ====== attention + out-proj partial + RS, per quarter =====
            def post_quarter(qd):
                """RS result -> +x residual -> LN1 (transpose issued apart)."""
                hpre = hpp.tile([P, D], F32, tag="hpre", name=f"hpre_{qd}")
                nc.sync.dma_start(hpre[:], reds[qd][:])
                nc.vector.tensor_tensor(hpre[:], hpre[:], xq_t[:, qd, :],
                                        ALU.add)
                _layernorm(nc, lnp, h_t[:, qd, :], hpre[:], g1b_t, be1b_t,
                           eps_t, zero_t, affine)
                nc.vector.tensor_scalar_mul(hb_t[:, qd, :], h_t[:, qd, :],
                                            WS2)
                nc.vector.tensor_tensor(hb_t[:, qd, :], hb_t[:, qd, :],
                                        b2b_t[:], ALU.add)

            def pe_transpose(qd):
                """h_t[:, qd, :] -> hT_t[:, :, qd*P:...] via 8 PE transposes
                through a pGH psum tile, drained by the Pool engine."""
                trp = pt("pGH", f"tr_{qd}")
                for dc in range(DC):
                    nc.tensor.transpose(trp[:, dc * P:(dc + 1) * P],
                                        h_t[:, qd, dc * P:(dc + 1) * P],
                                        ident[:])
                nc.vector.tensor_copy(
                    hT_t[:, :, qd * P:(qd + 1) * P],
                    trp[:].rearrange("p (c t) -> p c t", t=P))

            for qd in range(QD):
                qcols = slice(qd * QL, (qd + 1) * QL)
                # ---- attention for 2 pairs ----
                stprot = ["pAB", "pCD"]
                sti = 0
                for pair in range(NP):
                    pvs = [pt("pE", f"pv_{qd}_{pair}_0", cols=512),
                           pt("pF", f"pv_{qd}_{pair}_1", cols=512)]
                    for ktp in range(KT // 2):
                        # S for both heads first, then both PVs; the stp psum
                        # rotates over 3 double-bank slots so the PE can run
                        # several S groups ahead of the ACT exp stream.
                        ptts = []
                        for h2 in range(2):
                            rows = slice(h2 * HD, h2 * HD + HD)
                            stp = pt(stprot[sti % 2],
                                     f"st_{qd}_{pair}_{ktp}_{h2}")
                            sti += 1
                            for j in range(2):
                                kt = 2 * ktp + j
                                nc.tensor.matmul(
                                    stp[:, j * 512:(j + 1) * 512],
                                    kT_t[rows, pair, kt * P:(kt + 1) * P],
                                    qT_t[rows, pair, qcols],
                                    start=True, stop=True)
                            ptt = atp.tile([P, 2, 512], BF16, tag="pt",
                                           name=f"pt_{qd}_{pair}_{ktp}_{h2}")
                            nc.scalar.activation(
                                ptt[:],
                                stp[:].rearrange("p (c n) -> p c n", n=512),
                                AF.Exp, scale=0.125 / WS2)
                            ptts.append(ptt)
                        for h2 in range(2):
                            for j in range(2):
                                kt = 2 * ktp + j
                                nc.tensor.matmul(
                                    pvs[h2][:HD + 1, :],
                                    v_aug[:, kt, 2 * pair + h2, :],
                                    ptts[h2][:, j, :],
                                    start=(kt == 0), stop=(kt == KT - 1))
                    for h2 in range(2):
                        rows = slice(h2 * HD, h2 * HD + HD)
                        den = atd.tile([1, QL], F32, tag="den",
                                       name=f"den_{qd}_{pair}_{h2}")
                        nc.vector.reciprocal(den[:], pvs[h2][HD:HD + 1, :])
                        denb = atd.tile([HD, QL], F32, tag="denb",
                                        name=f"denb_{qd}_{pair}_{h2}")
                        nc.gpsimd.partition_broadcast(denb[:], den[:])
                        nc.vector.tensor_tensor(outSB[rows, pair, :],
                                                pvs[h2][:HD, :], denb[:],
                                                ALU.mult)

                # ---- out-proj partial: [512 q, 1024], psum->SBUF->DRAM ----
                poslots = ["pGH", "pAB", "pCD", "pGH"]
                for tt4 in range(4):
                    po = pt(poslots[tt4], f"po_{qd}_{tt4}")
                    for pair in range(NP):
                        for oc in range(2):
                            nc.tensor.matmul(
                                po[:, oc * 512:(oc + 1) * 512],
                                outSB[:, pair, tt4 * P:(tt4 + 1) * P],
                                wo_t[:, pair, oc * 512:(oc + 1) * 512],
                                start=(pair == 0), stop=(pair == NP - 1))
                    pst = stg.tile([P, D], F32, tag="pstg",
                                   name=f"pstg_{qd}_{tt4}")
                    # drain on both DVE and ACT: ACT has a lull at quarter
                    # end while the next quarter's S matmuls prime the exps
                    if tt4 % 2 == 0:
                        nc.vector.tensor_copy(pst[:], po[:])
                    else:
                        nc.scalar.activation(pst[:], po[:], AF.Copy)
                    nc.sync.dma_start(
                        parts[qd].rearrange("(a p) d -> p a d", p=P)[:, tt4, :],
                        pst[:])
                nc.gpsimd.collective_compute(
                    "ReduceScatter", ALU.add, replica_groups=GROUPS,
                    ins=[parts[qd][:].opt()], outs=[reds[qd][:].opt()])
            # posts AFTER the full attention loop, with scheduler hints so
            # the greedy per-engine scheduler does not slot their long
            # collective-gated waits into the attention instruction streams
            # (a wait at a queue head blocks everything behind it).
            for qd in range(QD):
                with tc.tile_wait_until((0.138, 0.185, 0.235, 0.285)[qd]):
                    post_quarter(qd)
            for qd in range(QD - 1):
                pe_transpose(qd)

            # ================= FFN =====================================
            # Four phases: fc1(c1) -> fc2(c1) -> fc1(c2) -> fc2(c2).
            # fc2(c1) only needs ff1 for chunk-1 tokens, so it covers the
            # RS_q3 -> LN1_q3 -> transpose latency that gates fc1(c2).
            # w1 and w2 each stream twice (bf16, ~270 GB/s bursts).
            ff1 = pers.tile([P, FC, QD * P], F8, tag="blobA")

            def fc1_chunk(ch):
                tcols = slice(ch * 256, (ch + 1) * 256)
                f1AB = pt("pAB", f"f1AB_{ch}")
                f1CD = pt("pCD", f"f1CD_{ch}")
                f1slots = [(f1AB, 0), (f1AB, 512), (f1CD, 0), (f1CD, 512)]
                for fc4 in range(FC // 4):
                    w1_t = fp.tile([P, 4, 4, 2, P], F8, tag="w1s")
                    nc.sync.dma_start(w1_t[:], w1[fc4])
                    for f in range(4):
                        fc = 4 * fc4 + f
                        tile_, sl = f1slots[f]
                        for dcp in range(4):
                            nc.tensor.matmul(
                                tile_[:, sl:sl + 256],
                                w1_t[:, f, dcp],
                                hT_t[:, 2 * dcp:2 * dcp + 2, tcols],
                                start=(dcp == 0), stop=(dcp == 3),
                                perf_mode=mybir.MatmulPerfMode.DoubleRow)
                        nc.vector.tensor_scalar(
                            ff1[:, fc, tcols], tile_[:, sl:sl + 256],
                            b1_t[:, fc:fc + 1], 0.0, ALU.add, ALU.max)

            def fc2_chunk(ch):
                tts = (2 * ch, 2 * ch + 1)
                tGH = pt("pGH", f"f2GH_{ch}")
                tE = pt("pE", f"f2E_{ch}", cols=512)
                tF = pt("pF", f"f2F_{ch}", cols=512)
                dsts = {(tts[0], 0): tGH[:, 0:512], (tts[0], 1): tGH[:, 512:1024],
                        (tts[1], 0): tE[:, :], (tts[1], 1): tF[:, :]}
                for fcp in range(FC // 2):
                    w2_t = fp2.tile([P, 2, D], F8, tag="w2s")
                    nc.sync.dma_start(w2_t[:], w2[fcp])
                    st = (fcp == 0)
                    sp = (fcp == FC // 2 - 1)
                    for tt in tts:
                        for oc in range(2):
                            nc.tensor.matmul(
                                dsts[(tt, oc)],
                                ff1[:, 2 * fcp:2 * fcp + 2,
                                    tt * P:(tt + 1) * P],
                                w2_t[:, :, oc * 512:(oc + 1) * 512],
                                start=st, stop=sp,
                                perf_mode=mybir.MatmulPerfMode.DoubleRow)
                for i, tt in enumerate(tts):
                    t2 = t2p.tile([P, D], F32, tag="t2", name=f"t2_{tt}")
                    if i == 0:
                        nc.vector.tensor_tensor(t2[:], tGH[:],
                                                hb_t[:, tt, :], ALU.add)
                    else:
                        nc.vector.tensor_tensor(t2[:, 0:512], tE[:],
                                                hb_t[:, tt, 0:512], ALU.add)
                        nc.vector.tensor_tensor(t2[:, 512:1024], tF[:],
                                                hb_t[:, tt, 512:1024], ALU.add)
                    _layernorm(nc, lnp, t2[:], t2[:], g2b_t, be2b_t, eps_t,
                               zero_t, affine)
                    nc.sync.dma_start(y[tt], t2[:])

            fc1_chunk(0)
            fc2_chunk(0)
            pe_transpose(QD - 1)
            fc1_chunk(1)
            fc2_chunk(1)

    nc.compile()
    return nc


def make_in_maps(x, w_qkv, b_qkv, w_o, b_o, g1, be1, w1, b1, w2, b2, g2, be2):
    f = np.float32
    x = np.asarray(x, f)
    w_qkv = np.asarray(w_qkv, f)
    b_qkv = np.asarray(b_qkv, f)
    w_o = np.asarray(w_o, f)
    w1 = np.asarray(w1, f)
    w2 = np.asarray(w2, f)

    bc = lambda v, n=D: np.ascontiguousarray(
        np.broadcast_to(np.asarray(v, f).reshape(1, n), (P, n)))
    # w1/w2 in streaming-chunk SBUF layout
    w1h = np.ascontiguousarray(
        (w1 * WS).reshape(DC, P, FC, P).transpose(2, 1, 0, 3)  # [fc, k, dc, m]
        .reshape(FC // 4, 4, P, 4, 2, P).transpose(0, 2, 1, 3, 4, 5)
        .astype(F8NP))
    w2h = np.ascontiguousarray(
        (w2 * WS).reshape(FC, P, D).reshape(FC // 2, 2, P, D)
        .transpose(0, 2, 1, 3).astype(F8NP))
    shared = {
        "w1": w1h, "w2": w2h,
        "b1": np.ascontiguousarray(np.asarray(b1, f).reshape(FC, P).T * WS),
        "b2b": bc(np.asarray(b2, f) * WS2),
        "g1b": bc(g1), "be1b": bc(be1), "g2b": bc(g2), "be2b": bc(be2),
        "vones": np.full((P, KT), WS, f),
    }
    in_maps = []
    for c in range(8):
        n, r = divmod(c, 4)
        cols = slice(r * 256, (r + 1) * 256)
        xTn = np.ascontiguousarray(
            x[n].T.reshape(DC, P, L).transpose(1, 0, 2).astype(F8NP))
        rows = np.concatenate(
            [np.arange(q * QL + r * P, q * QL + (r + 1) * P)
             for q in range(QD)])
        xqn = np.ascontiguousarray(
            (x[n][rows] + np.asarray(b_o, f).reshape(1, D))
            .reshape(QD, P, D).transpose(1, 0, 2).astype(BFNP))
        m = dict(shared)
        m["xT"] = xTn
        m["xq"] = xqn
        m["wq"] = np.ascontiguousarray(
            (w_qkv[:, :D][:, cols] * WS).reshape(4, 2, P, NP, P)
            .transpose(2, 3, 0, 1, 4).astype(F8NP))
        m["wk"] = np.ascontiguousarray(
            (w_qkv[:, D:2 * D][:, cols] * WS).reshape(4, 2, P, NP, P)
            .transpose(2, 3, 0, 1, 4).astype(F8NP))
        m["wv"] = np.ascontiguousarray(
            (w_qkv[:, 2 * D:][:, cols] * WS).reshape(4, 2, P, 2 * P)
            .transpose(2, 0, 1, 3).astype(F8NP))
        m["wo"] = np.ascontiguousarray(
            w_o[cols, :].reshape(NP, P, D).transpose(1, 0, 2).astype(BFNP))
        m["bq"] = np.ascontiguousarray(
            b_qkv[:D][cols].reshape(NP, P).T * WS)
        m["bk"] = np.ascontiguousarray(
            b_qkv[D:2 * D][cols].reshape(NP, P).T * WS)
        m["bv"] = bc(b_qkv[2 * D:][cols] * WS, 2 * P)
        in_maps.append(m)
    return in_maps


def get_nc(affine=True):
    if affine not in _CACHED_NC:
        _CACHED_NC[affine] = _build_nc(affine)
    return _CACHED_NC[affine]


def kernel(**inputs):
    in_maps = make_in_maps(**inputs)
    affine = not (np.all(np.asarray(inputs["g1"]) == 1)
                  and np.all(np.asarray(inputs["be1"]) == 0)
                  and np.all(np.asarray(inputs["g2"]) == 1)
                  and np.all(np.asarray(inputs["be2"]) == 0))
    nc = get_nc(affine)
    # The axon-proxied NRT occasionally reports a transient
    # NRT_EXEC_UNIT_UNRECOVERABLE on a cold first dispatch; a plain retry
    # has always succeeded with bit-identical results, so recover inline.
    last_err = None
    for _ in range(3):
        try:
            res = run_bass_kernel_spmd(nc, in_maps, list(range(8))).results
            break
        except Exception as e:  # noqa: BLE001
            last_err = e
    else:
        raise last_err
    out = np.empty((NB, L, D), np.float32)
    for c in range(8):
        n, r = divmod(c, 4)
        yv = res[c]["y"]
        for q in range(QD):
            out[n, q * QL + r * P:q * QL + (r + 1) * P] = yv[q]
    return out


if __name__ == "__main__":
    rng = np.random.default_rng(0)
    demo = {
        "x": rng.standard_normal((NB, L, D)).astype(np.float32),
        "w_qkv": rng.standard_normal((D, 3 * D)).astype(np.float32) * 0.03,
        "b_qkv": rng.standard_normal(3 * D).astype(np.float32) * 0.03,
        "w_o": rng.standard_normal((D, D)).astype(np.float32) * 0.03,
        "b_o": rng.standard_normal(D).astype(np.float32) * 0.03,
        "g1": np.ones(D, np.float32), "be1": np.zeros(D, np.float32),
        "w1": rng.standard_normal((D, FF)).astype(np.float32) * 0.03,
        "b1": rng.standard_normal(FF).astype(np.float32) * 0.03,
        "w2": rng.standard_normal((FF, D)).astype(np.float32) * 0.015,
        "b2": rng.standard_normal(D).astype(np.float32) * 0.015,
        "g2": np.ones(D, np.float32), "be2": np.zeros(D, np.float32),
    }
    out = kernel(**demo)
    print("kernel output:", out.shape, out.dtype, np.abs(out).mean())


# revision 36
# speedup vs baseline: 1.3062x; 1.1338x over previous
"""Trainium2 Bass kernel for a transformer encoder layer (nn_Encoder).

x:[2,2048,1024] f32, 8 NeuronCores. Hybrid data/tensor parallel: core c
handles batch n=c//4 and head-group r=c%4 (4 of 16 heads). Each core
computes Q/K/V for its 4 heads over all 2048 tokens (no K/V recompute
redundancy), runs attention per 512-query quarter, then a partial output
projection; a per-quarter ReduceScatter over the 4-core group sums the
partials and hands each core 128 tokens per quarter (512 total) for the
LN1+FFN+LN2 tail. The 4 ReduceScatters run on the collective cores,
overlapped with attention of later quarters.

Matmul operands are bf16 (same PE rate as f32r, half the DMA/SBUF);
accumulation and the collective stay f32. LN stats in f32.
"""
import os
import sys

for _p in ("/opt/trn_rl_repo", "/root/.axon_site/_ro/trn_rl_repo"):
    if os.path.isdir(_p) and _p not in sys.path:
        sys.path.insert(0, _p)

import numpy as np
import ml_dtypes
import concourse.bass as bass
import concourse.mybir as mybir
import concourse.tile as tile
from concourse import bacc
from concourse.bass_utils import run_bass_kernel_spmd
from concourse.masks import make_identity

F32 = mybir.dt.float32
BF16 = mybir.dt.bfloat16
F8 = mybir.dt.float8e4
AF = mybir.ActivationFunctionType
ALU = mybir.AluOpType
BFNP = ml_dtypes.bfloat16
F8NP = mybir.dt.np(F8)
WS = 64.0           # host scale on w1/w2 so fp8e4m3 stays in normal range
WS2 = WS * WS       # folded into hb; LN2 is scale-invariant

D = 1024
H = 16
HD = 64
FF = 4096
L = 2048
NB = 2
P = 128
DC = D // P       # 8 chunks of the model dim
KT = L // P       # 16 key tiles
FC = FF // P      # 32 ff chunks
NP = 2            # head pairs per core (4 heads)
QD = 4            # query quarters
QL = L // QD      # 512 queries per quarter
EPS = 1e-5
GROUPS = [[0, 1, 2, 3], [4, 5, 6, 7]]

_CACHED_NC = {}


def _layernorm(nc, pool, dst, src, g_t, be_t, eps_t, zero_t, affine):
    """dst = (src - mean)/sqrt(var + eps) [* g + be], row-wise over 1024.

    var = E[x^2] - mu^2 (safe here: |mu| << rms). One Newton step refines
    the reciprocal sqrt.
    """
    mu = pool.tile([P, 1], F32, tag="ln_mu")
    nc.vector.tensor_reduce(mu[:], src, mybir.AxisListType.X, ALU.add)
    nc.vector.tensor_scalar_mul(mu[:], mu[:], 1.0 / D)
    c = pool.tile([P, D], F32, tag="ln_c")
    ss = pool.tile([P, 1], F32, tag="ln_ss")
    nc.scalar.activation(c[:], src, AF.Square, accum_out=ss[:])
    vv = pool.tile([P, 1], F32, tag="ln_v")
    nc.vector.tensor_scalar(vv[:], ss[:], 1.0 / D, EPS, ALU.mult, ALU.add)
    m2 = pool.tile([P, 1], F32, tag="ln_m2")
    nc.vector.tensor_tensor(m2[:], mu[:], mu[:], ALU.mult)
    nc.vector.tensor_tensor(vv[:], vv[:], m2[:], ALU.subtract)
    s = pool.tile([P, 1], F32, tag="ln_s")
    nc.scalar.activation(s[:], vv[:], AF.Sqrt, bias=zero_t[:])
    r = pool.tile([P, 1], F32, tag="ln_r")
    nc.vector.reciprocal(r[:], s[:])
    t = pool.tile([P, 1], F32, tag="ln_t")
    nc.vector.tensor_tensor(t[:], r[:], r[:], ALU.mult)
    nc.vector.tensor_tensor(t[:], t[:], vv[:], ALU.mult)
    nc.vector.tensor_scalar(t[:], t[:], -0.5, 1.5, ALU.mult, ALU.add)
    nc.vector.tensor_tensor(r[:], r[:], t[:], ALU.mult)
    nc.vector.tensor_scalar(dst, src, mu[:], r[:], ALU.subtract, ALU.mult)
    if affine:
        nc.vector.tensor_tensor(dst, dst, g_t[:], ALU.mult)
        nc.vector.tensor_tensor(dst, dst, be_t[:], ALU.add)


def _build_nc(affine=True):
    nc = bacc.Bacc("TRN2", target_bir_lowering=False, num_devices=8)

    def dparam(name, shape, dt=BF16):
        return nc.dram_tensor(name, shape, dt, kind="ExternalInput")

    xT = dparam("xT", [P, DC, L], F8)       # x[n].T as [p, dc, t] (d=dc*128+p)
    xq = dparam("xq", [P, QD, D])           # owned token tiles, + b_o folded
    wq = dparam("wq", [P, NP, 4, 2, P], F8)  # [dpart, pair, dcp, slab, qcols]
    wk = dparam("wk", [P, NP, 4, 2, P], F8)
    wv = dparam("wv", [P, 4, 2, 2 * P], F8)  # [dpart, dcp, slab, vcols]
    wo = dparam("wo", [P, NP, D])           # [hd-part, pair, ocols]
    w1 = dparam("w1", [FC // 4, P, 4, 4, 2, P], F8)  # fc4-chunk, dcp-paired
    w2 = dparam("w2", [FC // 4, P, 2, 2, D], F8)     # 2 fcp per chunk
    bq = dparam("bq", [P, NP], F32)
    bk = dparam("bk", [P, NP], F32)
    bv = dparam("bv", [P, 2 * P], F32)      # per-column bias, broadcast rows
    b1 = dparam("b1", [P, FC], F32)
    b2b = dparam("b2b", [P, D], F32)
    g1b = dparam("g1b", [P, D], F32)
    be1b = dparam("be1b", [P, D], F32)
    g2b = dparam("g2b", [P, D], F32)
    be2b = dparam("be2b", [P, D], F32)
    vones = dparam("vones", [P, KT], F32)

    parts = [nc.dram_tensor(f"part{q}", [QL, D], F32) for q in range(QD)]
    reds = [nc.dram_tensor(f"red{q}", [P, D], F32) for q in range(QD)]
    y = nc.dram_tensor("y", [QD, P, D], F32, kind="ExternalOutput")

    with tile.TileContext(nc) as tc:
        with tc.tile_pool(name="pers", bufs=1) as pers, \
             tc.tile_pool(name="wp", bufs=1) as wp, \
             tc.tile_pool(name="stg", bufs=2) as stg, \
             tc.tile_pool(name="atp", bufs=3) as atp, \
             tc.tile_pool(name="atd", bufs=2) as atd, \
             tc.tile_pool(name="hpp", bufs=2) as hpp, \
             tc.tile_pool(name="lnp", bufs=2) as lnp, \
             tc.tile_pool(name="fp", bufs=2) as fp, \
             tc.tile_pool(name="fp2", bufs=2) as fp2, \
             tc.tile_pool(name="t2p", bufs=2) as t2p, \
             tc.tile_pool(name="ps", bufs=1, space="PSUM") as ps:

            # ---- persistent SBUF ----
            xT_t = pers.tile([P, DC, L], F8, tag="blobA")        # 16KB
            qT_t = pers.tile([P, NP, L], BF16, tag="qT")
            kT_t = pers.tile([P, NP, L], BF16, tag="kT")
            v_aug = pers.tile([P, KT, 4, HD + 1], BF16, tag="vaug")
            outSB = pers.tile([P, NP, QL], BF16, tag="outSB")
            h_t = pers.tile([P, QD, D], F32, tag="h")            # post-LN1
            hb_t = pers.tile([P, QD, D], F32, tag="hb")          # h + b2
            hT_t = pers.tile([P, DC, QD * P], F8, tag="hT")
            xq_t = pers.tile([P, QD, D], BF16, tag="xq")
            eps_t = pers.tile([P, 1], F32, tag="eps")
            zero_t = pers.tile([P, 1], F32, tag="zero")
            ident = pers.tile([P, P], F32, tag="ident")
            nc.gpsimd.memset(eps_t[:], EPS)
            nc.gpsimd.memset(zero_t[:], 0.0)
            make_identity(nc, ident[:])

            # psum tiles: fixed 8-bank plan, manually assigned per phase
            def pt(tag, name, cols=1024):
                return ps.tile([P, cols], F32, tag=tag, name=name)

            # ---- weight / bias prefetch (scalar queue) ----
            wq_t = wp.tile([P, NP, 4, 2, P], F8, tag="wq")
            wk_t = wp.tile([P, NP, 4, 2, P], F8, tag="wk")
            wv_t = wp.tile([P, 4, 2, 2 * P], F8, tag="wv")
            wo_t = wp.tile([P, NP, D], BF16, tag="wo")
            bq_t = wp.tile([P, NP], F32, tag="bq")
            bk_t = wp.tile([P, NP], F32, tag="bk")
            bv_t = wp.tile([P, 2 * P], F32, tag="bv")
            b1_t = wp.tile([P, FC], F32, tag="b1")
            b2b_t = wp.tile([P, D], F32, tag="b2b")
            ones_t = wp.tile([P, KT], F32, tag="ones")
            # wq pair0 first so the very first Q matmul can start sooner
            nc.scalar.dma_start(wq_t[:, 0], wq[:, 0])
            nc.scalar.dma_start(wq_t[:, 1], wq[:, 1])
            nc.scalar.dma_start(wk_t[:], wk[:])
            nc.scalar.dma_start(wv_t[:], wv[:])
            nc.scalar.dma_start(wo_t[:], wo[:])
            nc.scalar.dma_start(bq_t[:], bq[:])
            nc.scalar.dma_start(bk_t[:], bk[:])
            nc.scalar.dma_start(bv_t[:], bv[:])
            nc.scalar.dma_start(b1_t[:], b1[:])
            nc.scalar.dma_start(b2b_t[:], b2b[:])
            nc.scalar.dma_start(ones_t[:], vones[:])
            if affine:
                g1b_t = wp.tile([P, D], F32, tag="g1b")
                be1b_t = wp.tile([P, D], F32, tag="be1b")
                g2b_t = wp.tile([P, D], F32, tag="g2b")
                be2b_t = wp.tile([P, D], F32, tag="be2b")
                nc.scalar.dma_start(g1b_t[:], g1b[:])
                nc.scalar.dma_start(be1b_t[:], be1b[:])
                nc.scalar.dma_start(g2b_t[:], g2b[:])
                nc.scalar.dma_start(be2b_t[:], be2b[:])
            else:
                g1b_t = be1b_t = g2b_t = be2b_t = None

            # x stream (sync queue), per-dc so Q proj starts on chunk 0
            for dc in range(DC):
                nc.sync.dma_start(xT_t[:, dc, :], xT[:, dc, :])
            nc.sync.dma_start(xq_t[:], xq[:])

            # 8 half-bank accumulation slots for the projection phases.
            # One tile per tag per phase; slots address halves explicitly
            # (repeated same-tag tile creation would WAW-serialize).
            def phase_slots(phase):
                tiles = {t: pt(t, f"{phase}_{t}",
                               cols=512 if t in ("pE", "pF") else 1024)
                         for t in ("pAB", "pCD", "pGH", "pE", "pF")}
                layout = [("pAB", 0), ("pAB", 512), ("pCD", 0), ("pCD", 512),
                          ("pGH", 0), ("pGH", 512), ("pE", 0), ("pF", 0)]
                return [(tiles[t], off) for t, off in layout]

            # ================= Q projection (dc-outer sweeps) ==========
            # dc-outer so the first matmuls ride the xT chunk stream.
            qsl = phase_slots("q")
            for pair in range(NP):
                for dcp in range(4):
                    for qt in range(4):
                        tile_, off = qsl[pair * 4 + qt]
                        nc.tensor.matmul(
                            tile_[:, off:off + 512],
                            wq_t[:, pair, dcp],
                            xT_t[:, 2 * dcp:2 * dcp + 2,
                                 qt * 512:(qt + 1) * 512],
                            start=(dcp == 0), stop=(dcp == 3),
                            perf_mode=mybir.MatmulPerfMode.DoubleRow)
                for qt in range(4):
                    tile_, off = qsl[pair * 4 + qt]
                    nc.vector.tensor_scalar(
                        qT_t[:, pair, qt * 512:(qt + 1) * 512],
                        tile_[:, off:off + 512],
                        bq_t[:, pair:pair + 1], None, ALU.add)

            # ================= K projection (dc-inner) =================
            ksl = phase_slots("k")
            for pair in range(NP):
                for qt in range(4):
                    i = pair * 4 + qt
                    tile_, off = ksl[i]
                    for dcp in range(4):
                        nc.tensor.matmul(
                            tile_[:, off:off + 512],
                            wk_t[:, pair, dcp],
                            xT_t[:, 2 * dcp:2 * dcp + 2,
                                 qt * 512:(qt + 1) * 512],
                            start=(dcp == 0), stop=(dcp == 3),
                            perf_mode=mybir.MatmulPerfMode.DoubleRow)
                    nc.vector.tensor_scalar(
                        kT_t[:, pair, qt * 512:(qt + 1) * 512],
                        tile_[:, off:off + 512],
                        bk_t[:, pair:pair + 1], None, ALU.add)

            # ================= V projection ============================
            nc.vector.tensor_copy(
                v_aug[:, :, :, HD],
                ones_t[:, :, None].to_broadcast([P, KT, 4]))
            vsl = phase_slots("v")
            for tt in range(KT):
                tile_, off = vsl[tt % 8]
                for dcp in range(4):
                    nc.tensor.matmul(
                        tile_[:, off:off + 256],
                        xT_t[:, 2 * dcp:2 * dcp + 2, tt * P:(tt + 1) * P],
                        wv_t[:, dcp],
                        start=(dcp == 0), stop=(dcp == 3),
                        perf_mode=mybir.MatmulPerfMode.DoubleRow)
                nc.vector.tensor_tensor(
                    v_aug[:, tt, :, 0:HD],
                    tile_[:, off:off + 256].rearrange("p (h c) -> p h c", c=HD),
                    bv_t[:].rearrange("p (h c) -> p h c", c=HD),
                    ALU.add)

            # ====== attention + out-proj partial + RS, per quarter =====
            def post_quarter(qd):
                """RS result -> +x residual -> LN1 (transpose issued apart)."""
                hpre = hpp.tile([P, D], F32, tag="hpre", name=f"hpre_{qd}")
                nc.sync.dma_start(hpre[:], reds[qd][:])
                nc.vector.tensor_tensor(hpre[:], hpre[:], xq_t[:, qd, :],
                                        ALU.add)
                _layernorm(nc, lnp, h_t[:, qd, :], hpre[:], g1b_t, be1b_t,
                           eps_t, zero_t, affine)
                nc.vector.tensor_scalar_mul(hb_t[:, qd, :], h_t[:, qd, :],
                                            WS2)
                nc.vector.tensor_tensor(hb_t[:, qd, :], hb_t[:, qd, :],
                                        b2b_t[:], ALU.add)

            def pe_transpose(qd):
                """h_t[:, qd, :] -> hT_t[:, :, qd*P:...] via 8 PE transposes
                through a pGH psum tile, drained by the Pool engine."""
                trp = pt("pGH", f"tr_{qd}")
                for dc in range(DC):
                    nc.tensor.transpose(trp[:, dc * P:(dc + 1) * P],
                                        h_t[:, qd, dc * P:(dc + 1) * P],
                                        ident[:])
                nc.vector.tensor_copy(
                    hT_t[:, :, qd * P:(qd + 1) * P],
                    trp[:].rearrange("p (c t) -> p c t", t=P))

            for qd in range(QD):
                qcols = slice(qd * QL, (qd + 1) * QL)
                # ---- attention for 2 pairs ----
                stprot = ["pAB", "pCD"]
                sti = 0
                for pair in range(NP):
                    pvs = [pt("pE", f"pv_{qd}_{pair}_0", cols=512),
                           pt("pF", f"pv_{qd}_{pair}_1", cols=512)]
                    for ktp in range(KT // 2):
                        # S for both heads first, then both PVs; the stp psum
                        # rotates over 3 double-bank slots so the PE can run
                        # several S groups ahead of the ACT exp stream.
                        ptts = []
                        for h2 in range(2):
                            rows = slice(h2 * HD, h2 * HD + HD)
                            stp = pt(stprot[sti % 2],
                                     f"st_{qd}_{pair}_{ktp}_{h2}")
                            sti += 1
                            for j in range(2):
                                kt = 2 * ktp + j
                                nc.tensor.matmul(
                                    stp[:, j * 512:(j + 1) * 512],
                                    kT_t[rows, pair, kt * P:(kt + 1) * P],
                                    qT_t[rows, pair, qcols],
                                    start=True, stop=True)
                            ptt = atp.tile([P, 2, 512], BF16, tag="pt",
                                           name=f"pt_{qd}_{pair}_{ktp}_{h2}")
                            nc.scalar.activation(
                                ptt[:],
                                stp[:].rearrange("p (c n) -> p c n", n=512),
                                AF.Exp, scale=0.125 / WS2)
                            ptts.append(ptt)
                        for h2 in range(2):
                            for j in range(2):
                                kt = 2 * ktp + j
                                nc.tensor.matmul(
                                    pvs[h2][:HD + 1, :],
                                    v_aug[:, kt, 2 * pair + h2, :],
                                    ptts[h2][:, j, :],
                                    start=(kt == 0), stop=(kt == KT - 1))
                    for h2 in range(2):
                        rows = slice(h2 * HD, h2 * HD + HD)
                        den = atd.tile([1, QL], F32, tag="den",
                                       name=f"den_{qd}_{pair}_{h2}")
                        nc.vector.reciprocal(den[:], pvs[h2][HD:HD + 1, :])
                        denb = atd.tile([HD, QL], F32, tag="denb",
                                        name=f"denb_{qd}_{pair}_{h2}")
                        nc.gpsimd.partition_broadcast(denb[:], den[:])
                        nc.vector.tensor_tensor(outSB[rows, pair, :],
                                                pvs[h2][:HD, :], denb[:],
                                                ALU.mult)

                # ---- out-proj partial: [512 q, 1024], psum->SBUF->DRAM ----
                poslots = ["pGH", "pAB", "pCD", "pGH"]
                for tt4 in range(4):
                    po = pt(poslots[tt4], f"po_{qd}_{tt4}")
                    for pair in range(NP):
                        for oc in range(2):
                            nc.tensor.matmul(
                                po[:, oc * 512:(oc + 1) * 512],
                                outSB[:, pair, tt4 * P:(tt4 + 1) * P],
                                wo_t[:, pair, oc * 512:(oc + 1) * 512],
                                start=(pair == 0), stop=(pair == NP - 1))
                    pst = stg.tile([P, D], F32, tag="pstg",
                                   name=f"pstg_{qd}_{tt4}")
                    # drain on both DVE and ACT: ACT has a lull at quarter
                    # end while the next quarter's S matmuls prime the exps
                    if tt4 % 2 == 0:
                        nc.vector.tensor_copy(pst[:], po[:])
                    else:
                        nc.scalar.activation(pst[:], po[:], AF.Copy)
                    nc.sync.dma_start(
                        parts[qd].rearrange("(a p) d -> p a d", p=P)[:, tt4, :],
                        pst[:])
                nc.gpsimd.collective_compute(
                    "ReduceScatter", ALU.add, replica_groups=GROUPS,
                    ins=[parts[qd][:].opt()], outs=[reds[qd][:].opt()])
            # posts AFTER the full attention loop, with scheduler hints so
            # the greedy per-engine scheduler does not slot their long
            # collective-gated waits into the attention instruction streams
            # (a wait at a queue head blocks everything behind it).
            for qd in range(QD):
                with tc.tile_wait_until((0.112, 0.158, 0.210, 0.262)[qd]):
                    post_quarter(qd)
            for qd in range(QD - 1):
                pe_transpose(qd)

            # ================= FFN =====================================
            # Four phases: fc1(c1) -> fc2(c1) -> fc1(c2) -> fc2(c2).
            # fc2(c1) only needs ff1 for chunk-1 tokens, so it covers the
            # RS_q3 -> LN1_q3 -> transpose latency that gates fc1(c2).
            # w1 and w2 each stream twice (bf16, ~270 GB/s bursts).
            ff1 = pers.tile([P, FC, QD * P], F8, tag="blobA")

            def fc1_chunk(ch):
                tcols = slice(ch * 256, (ch + 1) * 256)
                f1AB = pt("pAB", f"f1AB_{ch}")
                f1CD = pt("pCD", f"f1CD_{ch}")
                f1slots = [(f1AB, 0), (f1AB, 512), (f1CD, 0), (f1CD, 512)]
                for fc4 in range(FC // 4):
                    w1_t = fp.tile([P, 4, 4, 2, P], F8, tag="w1s")
                    nc.sync.dma_start(w1_t[:], w1[fc4])
                    for f in range(4):
                        fc = 4 * fc4 + f
                        tile_, sl = f1slots[f]
                        for dcp in range(4):
                            nc.tensor.matmul(
                                tile_[:, sl:sl + 256],
                                w1_t[:, f, dcp],
                                hT_t[:, 2 * dcp:2 * dcp + 2, tcols],
                                start=(dcp == 0), stop=(dcp == 3),
                                perf_mode=mybir.MatmulPerfMode.DoubleRow)
                        nc.vector.tensor_scalar(
                            ff1[:, fc, tcols], tile_[:, sl:sl + 256],
                            b1_t[:, fc:fc + 1], 0.0, ALU.add, ALU.max)

            def fc2_chunk(ch):
                tts = (2 * ch, 2 * ch + 1)
                tGH = pt("pGH", f"f2GH_{ch}")
                tE = pt("pE", f"f2E_{ch}", cols=512)
                tF = pt("pF", f"f2F_{ch}", cols=512)
                dsts = {(tts[0], 0): tGH[:, 0:512], (tts[0], 1): tGH[:, 512:1024],
                        (tts[1], 0): tE[:, :], (tts[1], 1): tF[:, :]}
                for fcg in range(FC // 4):
                    w2_t = fp2.tile([P, 2, 2, D], F8, tag="w2s")
                    nc.sync.dma_start(w2_t[:], w2[fcg])
                    for g in range(2):
                        fcp = 2 * fcg + g
                        st = (fcp == 0)
                        sp = (fcp == FC // 2 - 1)
                        for tt in tts:
                            for oc in range(2):
                                nc.tensor.matmul(
                                    dsts[(tt, oc)],
                                    ff1[:, 2 * fcp:2 * fcp + 2,
                                        tt * P:(tt + 1) * P],
                                    w2_t[:, g, :, oc * 512:(oc + 1) * 512],
                                    start=st, stop=sp,
                                    perf_mode=mybir.MatmulPerfMode.DoubleRow)
                for i, tt in enumerate(tts):
                    t2 = t2p.tile([P, D], F32, tag="t2", name=f"t2_{tt}")
                    if i == 0:
                        nc.vector.tensor_tensor(t2[:], tGH[:],
                                                hb_t[:, tt, :], ALU.add)
                    else:
                        nc.vector.tensor_tensor(t2[:, 0:512], tE[:],
                                                hb_t[:, tt, 0:512], ALU.add)
                        nc.vector.tensor_tensor(t2[:, 512:1024], tF[:],
                                                hb_t[:, tt, 512:1024], ALU.add)
                    _layernorm(nc, lnp, t2[:], t2[:], g2b_t, be2b_t, eps_t,
                               zero_t, affine)
                    nc.sync.dma_start(y[tt], t2[:])

            fc1_chunk(0)
            fc2_chunk(0)
            pe_transpose(QD - 1)
            fc1_chunk(1)
            fc2_chunk(1)

    nc.compile()
    return nc


def make_in_maps(x, w_qkv, b_qkv, w_o, b_o, g1, be1, w1, b1, w2, b2, g2, be2):
    f = np.float32
    x = np.asarray(x, f)
    w_qkv = np.asarray(w_qkv, f)
    b_qkv = np.asarray(b_qkv, f)
    w_o = np.asarray(w_o, f)
    w1 = np.asarray(w1, f)
    w2 = np.asarray(w2, f)

    bc = lambda v, n=D: np.ascontiguousarray(
        np.broadcast_to(np.asarray(v, f).reshape(1, n), (P, n)))
    # w1/w2 in streaming-chunk SBUF layout
    w1h = np.ascontiguousarray(
        (w1 * WS).reshape(DC, P, FC, P).transpose(2, 1, 0, 3)  # [fc, k, dc, m]
        .reshape(FC // 4, 4, P, 4, 2, P).transpose(0, 2, 1, 3, 4, 5)
        .astype(F8NP))
    w2h = np.ascontiguousarray(
        (w2 * WS).reshape(FC, P, D).reshape(FC // 4, 2, 2, P, D)
        .transpose(0, 3, 1, 2, 4).astype(F8NP))
    shared = {
        "w1": w1h, "w2": w2h,
        "b1": np.ascontiguousarray(np.asarray(b1, f).reshape(FC, P).T * WS),
        "b2b": bc(np.asarray(b2, f) * WS2),
        "g1b": bc(g1), "be1b": bc(be1), "g2b": bc(g2), "be2b": bc(be2),
        "vones": np.full((P, KT), WS, f),
    }
    in_maps = []
    for c in range(8):
        n, r = divmod(c, 4)
        cols = slice(r * 256, (r + 1) * 256)
        xTn = np.ascontiguousarray(
            x[n].T.reshape(DC, P, L).transpose(1, 0, 2).astype(F8NP))
        rows = np.concatenate(
            [np.arange(q * QL + r * P, q * QL + (r + 1) * P)
             for q in range(QD)])
        xqn = np.ascontiguousarray(
            (x[n][rows] + np.asarray(b_o, f).reshape(1, D))
            .reshape(QD, P, D).transpose(1, 0, 2).astype(BFNP))
        m = dict(shared)
        m["xT"] = xTn
        m["xq"] = xqn
        m["wq"] = np.ascontiguousarray(
            (w_qkv[:, :D][:, cols] * WS).reshape(4, 2, P, NP, P)
            .transpose(2, 3, 0, 1, 4).astype(F8NP))
        m["wk"] = np.ascontiguousarray(
            (w_qkv[:, D:2 * D][:, cols] * WS).reshape(4, 2, P, NP, P)
            .transpose(2, 3, 0, 1, 4).astype(F8NP))
        m["wv"] = np.ascontiguousarray(
            (w_qkv[:, 2 * D:][:, cols] * WS).reshape(4, 2, P, 2 * P)
            .transpose(2, 0, 1, 3).astype(F8NP))
        m["wo"] = np.ascontiguousarray(
            w_o[cols, :].reshape(NP, P, D).transpose(1, 0, 2).astype(BFNP))
        m["bq"] = np.ascontiguousarray(
            b_qkv[:D][cols].reshape(NP, P).T * WS)
        m["bk"] = np.ascontiguousarray(
            b_qkv[D:2 * D][cols].reshape(NP, P).T * WS)
        m["bv"] = bc(b_qkv[2 * D:][cols] * WS, 2 * P)
        in_maps.append(m)
    return in_maps


def get_nc(affine=True):
    if affine not in _CACHED_NC:
        _CACHED_NC[affine] = _build_nc(affine)
    return _CACHED_NC[affine]


def kernel(**inputs):
    in_maps = make_in_maps(**inputs)
    affine = not (np.all(np.asarray(inputs["g1"]) == 1)
                  and np.all(np.asarray(inputs["be1"]) == 0)
                  and np.all(np.asarray(inputs["g2"]) == 1)
                  and np.all(np.asarray(inputs["be2"]) == 0))
    nc = get_nc(affine)
    # The axon-proxied NRT occasionally reports a transient
    # NRT_EXEC_UNIT_UNRECOVERABLE on a cold first dispatch; a plain retry
    # has always succeeded with bit-identical results, so recover inline.
    last_err = None
    for _ in range(3):
        try:
            res = run_bass_kernel_spmd(nc, in_maps, list(range(8))).results
            break
        except Exception as e:  # noqa: BLE001
            last_err = e
    else:
        raise last_err
    out = np.empty((NB, L, D), np.float32)
    for c in range(8):
        n, r = divmod(c, 4)
        yv = res[c]["y"]
        for q in range(QD):
            out[n, q * QL + r * P:q * QL + (r + 1) * P] = yv[q]
    return out


if __name__ == "__main__":
    rng = np.random.default_rng(0)
    demo = {
        "x": rng.standard_normal((NB, L, D)).astype(np.float32),
        "w_qkv": rng.standard_normal((D, 3 * D)).astype(np.float32) * 0.03,
        "b_qkv": rng.standard_normal(3 * D).astype(np.float32) * 0.03,
        "w_o": rng.standard_normal((D, D)).astype(np.float32) * 0.03,
        "b_o": rng.standard_normal(D).astype(np.float32) * 0.03,
        "g1": np.ones(D, np.float32), "be1": np.zeros(D, np.float32),
        "w1": rng.standard_normal((D, FF)).astype(np.float32) * 0.03,
        "b1": rng.standard_normal(FF).astype(np.float32) * 0.03,
        "w2": rng.standard_normal((FF, D)).astype(np.float32) * 0.015,
        "b2": rng.standard_normal(D).astype(np.float32) * 0.015,
        "g2": np.ones(D, np.float32), "be2": np.zeros(D, np.float32),
    }
    out = kernel(**demo)
    print("kernel output:", out.shape, out.dtype, np.abs(out).mean())
